# revision 9
# baseline (speedup 1.0000x reference)
"""E3nn interaction (gnn message passing) Bass kernel for 8 Trainium2 cores.

Strategy: edges are sorted by receiver and partitioned so core i owns the
segment-sum for nodes [2560*i, 2560*(i+1)).  Each core redundantly computes
the up-projected node table (fp16) into its own DRAM from host-pretransposed
features, then streams its edge chunks: indirect-gather of sender rows,
radial MLP on the tensor engine, fused per-edge tensor-product ops on
DVE/ACT/POOL, and a one-hot matmul scatter accumulating messages in PSUM.
Per 128-node tile the accumulator is transposed on PE and the final linear
is applied, writing the core's output rows directly.
"""
import math
import os
import numpy as np

N_NODES = 20000
N_EDGES = 200000
MUL = 128
P = 128
NCORES = 8
TILES_PER_CORE = 20
NODES_PER_CORE = TILES_PER_CORE * P          # 2560
NODE_PAD = NCORES * NODES_PER_CORE           # 20480
N_NODE_TILES = NODE_PAD // P                 # 160
N_RADIAL = 8
HIDDEN = 64

_CACHE = {}


def _build(c_prof):
    import concourse.bacc as bacc
    import concourse.bass as bass
    import concourse.tile as tile
    from concourse import mybir

    f16, f32, i32 = mybir.dt.float16, mybir.dt.float32, mybir.dt.int32
    MUL_ = mybir.AluOpType.mult
    ADD_ = mybir.AluOpType.add
    EQ_ = mybir.AluOpType.is_equal
    SILU = mybir.ActivationFunctionType.Silu
    COPY = mybir.ActivationFunctionType.Copy

    nch = sum(c_prof)
    ne_pad = nch * P

    nc = bacc.Bacc()
    nfT = nc.declare_dram_parameter("nfT", [512, NODE_PAD], f16, isOutput=False)
    wup = nc.declare_dram_parameter("wup", [P, 512], f16, isOutput=False)
    w1d = nc.declare_dram_parameter("w1d", [N_RADIAL, HIDDEN], f16, isOutput=False)
    w2d = nc.declare_dram_parameter("w2d", [HIDDEN, HIDDEN], f16, isOutput=False)
    w3d = nc.declare_dram_parameter("w3d", [HIDDEN, HIDDEN], f16, isOutput=False)
    w4d = nc.declare_dram_parameter("w4d", [HIDDEN, 512], f16, isOutput=False)
    wlind = nc.declare_dram_parameter("wlind", [P, 512], f16, isOutput=False)
    iotad = nc.declare_dram_parameter("iotad", [P, P], f32, isOutput=False)
    identd = nc.declare_dram_parameter("identd", [P, P], f16, isOutput=False)
    gidxd = nc.declare_dram_parameter("gidxd", [ne_pad, 1], i32, isOutput=False)
    rlocd = nc.declare_dram_parameter("rlocd", [ne_pad, 1], f32, isOutput=False)
    attrd = nc.declare_dram_parameter("attrd", [ne_pad, 4], f32, isOutput=False)
    eftd = nc.declare_dram_parameter("eftd", [N_RADIAL, ne_pad], f16, isOutput=False)
    outd = nc.declare_dram_parameter("outd", [NODES_PER_CORE, 512], f32, isOutput=True)

    with tile.TileContext(nc) as tc:
        with tc.tile_pool(name="const", bufs=1) as cp, \
             tc.tile_pool(name="dram", bufs=1, space="DRAM") as dp, \
             tc.tile_pool(name="upsb", bufs=4) as up_sb, \
             tc.tile_pool(name="edge", bufs=8) as ep, \
             tc.tile_pool(name="prod", bufs=6) as pp, \
             tc.tile_pool(name="flush", bufs=3) as fp, \
             tc.tile_pool(name="psA", bufs=1, space="PSUM") as psA, \
             tc.tile_pool(name="psW", bufs=2, space="PSUM") as psW, \
             tc.tile_pool(name="psH", bufs=2, space="PSUM") as psH, \
             tc.tile_pool(name="psF", bufs=1, space="PSUM") as psF:

            table = dp.tile([NODE_PAD, 512], f16)

            wup_t = cp.tile([P, 512], f16)
            nc.sync.dma_start(out=wup_t[:], in_=wup[:])
            w1_t = cp.tile([N_RADIAL, HIDDEN], f16)
            nc.sync.dma_start(out=w1_t[:], in_=w1d[:])
            w2_t = cp.tile([HIDDEN, HIDDEN], f16)
            nc.sync.dma_start(out=w2_t[:], in_=w2d[:])
            w3_t = cp.tile([HIDDEN, HIDDEN], f16)
            nc.sync.dma_start(out=w3_t[:], in_=w3d[:])
            w4_t = cp.tile([HIDDEN, 512], f16)
            nc.sync.dma_start(out=w4_t[:], in_=w4d[:])
            wlin_t = cp.tile([P, 512], f16)
            nc.sync.dma_start(out=wlin_t[:], in_=wlind[:])
            iota_t = cp.tile([P, P], f32)
            nc.sync.dma_start(out=iota_t[:], in_=iotad[:])
            ident_t = cp.tile([P, P], f16)
            nc.sync.dma_start(out=ident_t[:], in_=identd[:])
            zt = cp.tile([P, P], f16)
            nc.vector.memset(zt[:], 0.0)

            # ---- Phase A: up-projection table (all nodes, replicated) ----
            for nt in range(N_NODE_TILES):
                r0 = nt * P
                ups = psW.tile([P, 512], f32, tag="w512", name="ups")
                for b in range(4):
                    xT = up_sb.tile([P, P], f16, tag="xT")
                    nc.sync.dma_start(
                        out=xT[:], in_=nfT[b * P:(b + 1) * P, r0:r0 + P])
                    nc.tensor.matmul(
                        out=ups[:, b * P:(b + 1) * P], lhsT=xT[:],
                        rhs=wup_t[:, b * P:(b + 1) * P], start=True, stop=True)
                urow = up_sb.tile([P, 512], f16, tag="urow")
                if nt % 2 == 0:
                    nc.vector.tensor_copy(out=urow[:], in_=ups[:])
                else:
                    nc.scalar.copy(out=urow[:], in_=ups[:])
                nc.sync.dma_start(out=table[r0:r0 + P, :], in_=urow[:])

            # ---- Phase B: edge chunks ----
            ci_global = 0
            for t in range(TILES_PER_CORE):
                n_chunks = c_prof[t]
                acc = psA.tile([P, 1024], f32, tag="acc")
                nc.tensor.matmul(out=acc[:, 0:512], lhsT=zt[:], rhs=wup_t[:],
                                 start=True, stop=True, skip_group_check=True)
                nc.tensor.matmul(out=acc[:, 512:1024], lhsT=zt[:], rhs=wup_t[:],
                                 start=True, stop=True, skip_group_check=True)
                for ci in range(n_chunks):
                    e0 = ci_global * P
                    ci_global += 1
                    gidx = ep.tile([P, 1], i32, tag="gidx")
                    nc.sync.dma_start(out=gidx[:], in_=gidxd[e0:e0 + P, :])
                    rloc = ep.tile([P, 1], f32, tag="rloc")
                    nc.sync.dma_start(out=rloc[:], in_=rlocd[e0:e0 + P, :])
                    at = ep.tile([P, 4], f32, tag="at")
                    nc.sync.dma_start(out=at[:], in_=attrd[e0:e0 + P, :])
                    eft = ep.tile([N_RADIAL, P], f16, tag="eft")
                    nc.sync.dma_start(out=eft[:], in_=eftd[:, e0:e0 + P])
                    g = ep.tile([P, 512], f16, tag="g")
                    nc.gpsimd.indirect_dma_start(
                        out=g[:], out_offset=None, in_=table[:],
                        in_offset=bass.IndirectOffsetOnAxis(ap=gidx[:, :1], axis=0))

                    # radial MLP (PE + ACT silu)
                    hps = psH.tile([HIDDEN, 3 * P], f32, tag="hps")
                    nc.tensor.matmul(out=hps[:, 0:P], lhsT=w1_t[:], rhs=eft[:],
                                     start=True, stop=True)
                    h1 = pp.tile([HIDDEN, P], f16, tag="h1")
                    nc.scalar.activation(out=h1[:], in_=hps[:, 0:P], func=SILU)
                    nc.tensor.matmul(out=hps[:, P:2 * P], lhsT=w2_t[:], rhs=h1[:],
                                     start=True, stop=True)
                    h2 = pp.tile([HIDDEN, P], f16, tag="h2")
                    nc.scalar.activation(out=h2[:], in_=hps[:, P:2 * P], func=SILU)
                    nc.tensor.matmul(out=hps[:, 2 * P:3 * P], lhsT=w3_t[:], rhs=h2[:],
                                     start=True, stop=True)
                    h3 = pp.tile([HIDDEN, P], f16, tag="h3")
                    nc.scalar.activation(out=h3[:], in_=hps[:, 2 * P:3 * P], func=SILU)
                    tpw = psW.tile([P, 512], f32, tag="w512", name="tpw")
                    nc.tensor.matmul(out=tpw[:], lhsT=h3[:], rhs=w4_t[:],
                                     start=True, stop=True)
                    wt = pp.tile([P, 512], f16, tag="wt")   # w0|w1|w2|w3
                    if ci % 2 == 0:
                        nc.vector.tensor_copy(out=wt[:], in_=tpw[:])
                    else:
                        nc.scalar.copy(out=wt[:], in_=tpw[:])

                    # per-edge tensor product -> mji [128e, 1024]
                    # layout: [m0a | m0b | m1a(3) | m1b(3)]
                    y0 = at[:, 0:1]
                    mji = pp.tile([P, 1024], f16, tag="mji")
                    nc.vector.scalar_tensor_tensor(
                        out=mji[:, 0:P], in0=g[:, 0:P], scalar=y0, in1=wt[:, 0:P],
                        op0=MUL_, op1=MUL_)
                    sw2 = pp.tile([P, P], f16, tag="sw2")
                    nc.vector.tensor_mul(out=sw2[:], in0=g[:, 0:P],
                                         in1=wt[:, 2 * P:3 * P])
                    for m in range(3):
                        nc.scalar.activation(
                            out=mji[:, (2 + m) * P:(3 + m) * P], in_=sw2[:],
                            func=COPY, scale=at[:, 1 + m:2 + m])
                    w3y0 = pp.tile([P, P], f16, tag="w3y0")
                    nc.vector.tensor_scalar(
                        out=w3y0[:], in0=wt[:, 3 * P:4 * P], scalar1=y0,
                        scalar2=None, op0=MUL_)
                    for m in range(3):
                        nc.vector.tensor_mul(
                            out=mji[:, (5 + m) * P:(6 + m) * P],
                            in0=g[:, (1 + m) * P:(2 + m) * P], in1=w3y0[:])
                    # m0b: d = sum_m v_m*y1_m via broadcast-multiply + adds
                    vy = pp.tile([P, 3 * P], f16, tag="vy")
                    nc.vector.tensor_tensor(
                        out=vy[:].rearrange("p (m u) -> p m u", u=P),
                        in0=g[:, P:4 * P].rearrange("p (m u) -> p m u", u=P),
                        in1=at[:, 1:4].to_broadcast([P, 3, P]),
                        op=MUL_)
                    d01 = pp.tile([P, P], f16, tag="d01")
                    nc.vector.tensor_add(out=d01[:], in0=vy[:, 0:P],
                                         in1=vy[:, P:2 * P])
                    d2 = pp.tile([P, P], f16, tag="d2")
                    nc.gpsimd.tensor_add(out=d2[:], in0=d01[:], in1=vy[:, 2 * P:3 * P])
                    nc.gpsimd.tensor_mul(out=mji[:, P:2 * P], in0=d2[:],
                                         in1=wt[:, P:2 * P])

                    oh = pp.tile([P, P], f16, tag="oh")
                    nc.vector.tensor_scalar(
                        out=oh[:], in0=iota_t[:], scalar1=rloc[:, :1], scalar2=None,
                        op0=EQ_)

                    nc.tensor.matmul(out=acc[:, 0:512], lhsT=oh[:],
                                     rhs=mji[:, 0:512], start=False,
                                     stop=(ci == n_chunks - 1),
                                     skip_group_check=True)
                    nc.tensor.matmul(out=acc[:, 512:1024], lhsT=oh[:],
                                     rhs=mji[:, 512:1024], start=False,
                                     stop=(ci == n_chunks - 1),
                                     skip_group_check=True)

                # ---- flush node tile t ----
                msg = fp.tile([P, 1024], f16, tag="msg")
                nc.vector.tensor_copy(out=msg[:, 0:512], in_=acc[:, 0:512])
                nc.scalar.copy(out=msg[:, 512:1024], in_=acc[:, 512:1024])
                psT = psF.tile([P, 1024], f16, tag="psTfin", name="psT")
                for b in range(8):
                    nc.tensor.transpose(
                        out=psT[:, b * P:(b + 1) * P],
                        in_=msg[:, b * P:(b + 1) * P], identity=ident_t[:])
                msgT = fp.tile([P, 1024], f16, tag="msgT")
                nc.vector.tensor_copy(out=msgT[:, 0:512], in_=psT[:, 0:512])
                nc.scalar.copy(out=msgT[:, 512:1024], in_=psT[:, 512:1024])
                fin = psF.tile([P, 1024], f32, tag="psTfin", name="fin")
                nc.tensor.matmul(out=fin[:, 0:512], lhsT=zt[:], rhs=wup_t[:],
                                 start=True, stop=True, skip_group_check=True)
                nc.tensor.matmul(out=fin[:, 0:P], lhsT=msgT[:, 0:P],
                                 rhs=wlin_t[:, 0:P], start=False, stop=False,
                                 skip_group_check=True)
                nc.tensor.matmul(out=fin[:, 0:P], lhsT=msgT[:, P:2 * P],
                                 rhs=wlin_t[:, P:2 * P], start=False, stop=True,
                                 skip_group_check=True)
                for m in range(3):
                    nc.tensor.matmul(
                        out=fin[:, (1 + m) * P:(2 + m) * P],
                        lhsT=msgT[:, (2 + m) * P:(3 + m) * P],
                        rhs=wlin_t[:, 2 * P:3 * P], start=False, stop=False,
                        skip_group_check=True)
                    nc.tensor.matmul(
                        out=fin[:, (1 + m) * P:(2 + m) * P],
                        lhsT=msgT[:, (5 + m) * P:(6 + m) * P],
                        rhs=wlin_t[:, 3 * P:4 * P], start=False, stop=True,
                        skip_group_check=True)
                ot = fp.tile([P, 512], f32, tag="ot")
                nc.vector.tensor_copy(out=ot[:, 0:P], in_=fin[:, 0:P])
                for m in range(3):
                    dst = ot[:, P + m:512:3]
                    if m == 0:
                        nc.vector.tensor_copy(out=dst, in_=fin[:, P:2 * P])
                    elif m == 1:
                        nc.scalar.copy(out=dst, in_=fin[:, 2 * P:3 * P])
                    else:
                        nc.vector.tensor_copy(out=dst, in_=fin[:, 3 * P:4 * P])
                nc.sync.dma_start(out=outd[t * P:(t + 1) * P, :], in_=ot[:])

    nc.compile()
    return nc


def _host_prep(inputs):
    nf = np.asarray(inputs["node_feats"], dtype=np.float32)
    ea = np.asarray(inputs["edge_attrs"], dtype=np.float32)
    ef = np.asarray(inputs["edge_feats"], dtype=np.float32)
    snd = np.asarray(inputs["sender"]).astype(np.int64)
    rcv = np.asarray(inputs["receiver"]).astype(np.int64)

    inv = 1.0 / math.sqrt(MUL)
    inv2 = 1.0 / math.sqrt(2 * MUL)
    c = 1.0 / math.sqrt(MUL)
    c3 = 1.0 / math.sqrt(3.0 * MUL)

    # node feats fp16, transposed block-major: row b*128+ch, col n
    s = nf[:, :MUL]
    v = nf[:, MUL:].reshape(-1, MUL, 3)
    nfT = np.zeros((512, NODE_PAD), np.float16)
    nfT[0:128, :N_NODES] = s.T
    for m in range(3):
        nfT[128 * (1 + m):128 * (2 + m), :N_NODES] = v[:, :, m].T

    wup = np.zeros((P, 512), np.float16)
    wup[:, 0:128] = (np.asarray(inputs["W_up0"]) * inv).astype(np.float16)
    w_up1 = (np.asarray(inputs["W_up1"]) * inv).astype(np.float16)
    for m in range(3):
        wup[:, 128 * (1 + m):128 * (2 + m)] = w_up1
    w1 = (np.asarray(inputs["mlp_w1"]) / math.sqrt(N_RADIAL)).astype(np.float16)
    w2 = (np.asarray(inputs["mlp_w2"]) / math.sqrt(HIDDEN)).astype(np.float16)
    w3 = (np.asarray(inputs["mlp_w3"]) / math.sqrt(HIDDEN)).astype(np.float16)
    w4 = np.asarray(inputs["mlp_w4"]) / math.sqrt(HIDDEN)
    w4 = w4 * np.concatenate([np.full(128, c), np.full(128, c3),
                              np.full(128, c), np.full(128, c)])
    w4 = w4.astype(np.float16)
    wlin = np.zeros((P, 512), np.float16)
    lin0 = (np.asarray(inputs["W_lin0"]) * inv2 / 10.0).astype(np.float16)
    lin1 = (np.asarray(inputs["W_lin1"]) * inv2 / 10.0).astype(np.float16)
    wlin[:, 0:128] = lin0[:128]
    wlin[:, 128:256] = lin0[128:]
    wlin[:, 256:384] = lin1[:128]
    wlin[:, 384:512] = lin1[128:]

    iota = np.tile(np.arange(P, dtype=np.float32), (P, 1))
    ident = np.eye(P, dtype=np.float16)

    core_of = rcv // NODES_PER_CORE
    tile_of = (rcv % NODES_PER_CORE) // P
    sizes = np.zeros((NCORES, TILES_PER_CORE), np.int64)
    np.add.at(sizes, (core_of, tile_of), 1)
    c_prof = tuple(max(1, int(math.ceil(sizes[:, t].max() / P)))
                   for t in range(TILES_PER_CORE))
    nch = sum(c_prof)
    ne_pad = nch * P

    order = np.lexsort((rcv, tile_of, core_of))
    gidx_all = np.zeros((NCORES, ne_pad, 1), np.int32)
    rloc_all = np.zeros((NCORES, ne_pad, 1), np.float32)
    attr_all = np.zeros((NCORES, ne_pad, 4), np.float32)
    eft_all = np.zeros((NCORES, N_RADIAL, ne_pad), np.float16)

    starts = np.concatenate([[0], np.cumsum(np.asarray(c_prof)) * P])[:-1]
    flat_sizes = sizes.reshape(-1)
    run_start = np.concatenate([[0], np.cumsum(flat_sizes)])[:-1].reshape(
        NCORES, TILES_PER_CORE)

    for cidx in range(NCORES):
        for t in range(TILES_PER_CORE):
            n = int(sizes[cidx, t])
            if n == 0:
                continue
            e = order[run_start[cidx, t]:run_start[cidx, t] + n]
            s0 = int(starts[t])
            gidx_all[cidx, s0:s0 + n, 0] = snd[e]
            rloc_all[cidx, s0:s0 + n, 0] = (rcv[e] % NODES_PER_CORE) - t * P
            attr_all[cidx, s0:s0 + n, :] = ea[e]
            eft_all[cidx, :, s0:s0 + n] = ef[e].astype(np.float16).T

    common = dict(nfT=nfT, wup=wup, w1d=w1, w2d=w2, w3d=w3, w4d=w4,
                  wlind=wlin, iotad=iota, identd=ident)
    in_maps = []
    for cidx in range(NCORES):
        m = dict(common)
        m.update(gidxd=gidx_all[cidx], rlocd=rloc_all[cidx],
                 attrd=attr_all[cidx], eftd=eft_all[cidx])
        in_maps.append(m)
    return c_prof, in_maps


def kernel(**inputs):
    from concourse.bass_utils import run_bass_kernel_spmd

    c_prof, in_maps = _host_prep(inputs)
    if c_prof not in _CACHE:
        _CACHE[c_prof] = _build(c_prof)
    nc = _CACHE[c_prof]

    trace = bool(os.environ.get("KERNEL_TRACE"))
    if trace:
        import sys, types
        import concourse.bass_utils as bu
        try:
            import antenv.axon_hooks  # noqa
        except ImportError:
            import trn_agent_boot.trn_boot as tb
            hooks = types.ModuleType("antenv.axon_hooks")
            hk = tb._ntff_profile_via_ctypes("/opt/axon/libaxon_pjrt.so")
            hooks.get_axon_ntff_profile_hook = lambda: hk
            hooks.set_axon_ntff_profile_hook = lambda h: None
            sys.modules["antenv.axon_hooks"] = hooks
        bu.upload_artifacts = lambda d: d

    res = run_bass_kernel_spmd(nc, in_maps, list(range(NCORES)), trace=trace)
    if trace and res.exec_time_ns is not None:
        print(f"HW exec time: {res.exec_time_ns} ns")
        if res.instructions_and_trace:
            print(f"trace: {res.instructions_and_trace[1]}")

    out = np.empty((N_NODES, 512), np.float32)
    for cidx in range(NCORES):
        lo = cidx * NODES_PER_CORE
        hi = min((cidx + 1) * NODES_PER_CORE, N_NODES)
        if lo >= N_NODES:
            break
        out[lo:hi] = res.results[cidx]["outd"][:hi - lo]
    return out


# revision 10
# speedup vs baseline: 1.0208x; 1.0208x over previous
"""E3nn interaction (gnn message passing) Bass kernel for 8 Trainium2 cores.

Strategy: edges are sorted by receiver and partitioned so core i owns the
segment-sum for nodes [2560*i, 2560*(i+1)).  Each core redundantly computes
the up-projected node table (fp16) into its own DRAM from host-pretransposed
features, then streams its edge chunks: indirect-gather of sender rows,
radial MLP on the tensor engine, fused per-edge tensor-product ops on
DVE/ACT/POOL, and a one-hot matmul scatter accumulating messages in PSUM.
Per 128-node tile the accumulator is transposed on PE and the final linear
is applied, writing the core's output rows directly.
"""
import math
import os
import numpy as np

N_NODES = 20000
N_EDGES = 200000
MUL = 128
P = 128
NCORES = 8
TILES_PER_CORE = 20
NODES_PER_CORE = TILES_PER_CORE * P          # 2560
NODE_PAD = NCORES * NODES_PER_CORE           # 20480
N_NODE_TILES = NODE_PAD // P                 # 160
N_RADIAL = 8
HIDDEN = 64

_CACHE = {}


def _build(c_prof):
    import concourse.bacc as bacc
    import concourse.bass as bass
    import concourse.tile as tile
    from concourse import mybir

    f16, f32, i32 = mybir.dt.float16, mybir.dt.float32, mybir.dt.int32
    MUL_ = mybir.AluOpType.mult
    ADD_ = mybir.AluOpType.add
    EQ_ = mybir.AluOpType.is_equal
    SILU = mybir.ActivationFunctionType.Silu
    COPY = mybir.ActivationFunctionType.Copy

    nch = sum(c_prof)
    ne_pad = nch * P

    nc = bacc.Bacc()
    nfT = nc.declare_dram_parameter("nfT", [512, NODE_PAD], f16, isOutput=False)
    wup = nc.declare_dram_parameter("wup", [P, 512], f16, isOutput=False)
    w1d = nc.declare_dram_parameter("w1d", [N_RADIAL, HIDDEN], f16, isOutput=False)
    w2d = nc.declare_dram_parameter("w2d", [HIDDEN, HIDDEN], f16, isOutput=False)
    w3d = nc.declare_dram_parameter("w3d", [HIDDEN, HIDDEN], f16, isOutput=False)
    w4d = nc.declare_dram_parameter("w4d", [HIDDEN, 512], f16, isOutput=False)
    wlind = nc.declare_dram_parameter("wlind", [P, 512], f16, isOutput=False)
    iotad = nc.declare_dram_parameter("iotad", [P, P], f32, isOutput=False)
    identd = nc.declare_dram_parameter("identd", [P, P], f16, isOutput=False)
    gidxd = nc.declare_dram_parameter("gidxd", [ne_pad, 1], i32, isOutput=False)
    rlocd = nc.declare_dram_parameter("rlocd", [ne_pad, 1], f32, isOutput=False)
    attrd = nc.declare_dram_parameter("attrd", [ne_pad, 4], f32, isOutput=False)
    eftd = nc.declare_dram_parameter("eftd", [N_RADIAL, ne_pad], f16, isOutput=False)
    outd = nc.declare_dram_parameter("outd", [NODES_PER_CORE, 512], f32, isOutput=True)

    with tile.TileContext(nc) as tc:
        with tc.tile_pool(name="const", bufs=1) as cp, \
             tc.tile_pool(name="dram", bufs=1, space="DRAM") as dp, \
             tc.tile_pool(name="upsb", bufs=4) as up_sb, \
             tc.tile_pool(name="edge", bufs=8) as ep, \
             tc.tile_pool(name="prod", bufs=6) as pp, \
             tc.tile_pool(name="flush", bufs=3) as fp, \
             tc.tile_pool(name="psA", bufs=1, space="PSUM") as psA, \
             tc.tile_pool(name="psW", bufs=2, space="PSUM") as psW, \
             tc.tile_pool(name="psH", bufs=3, space="PSUM") as psH, \
             tc.tile_pool(name="psF", bufs=1, space="PSUM") as psF:

            table = dp.tile([NODE_PAD, 512], f16)

            wup_t = cp.tile([P, 512], f16)
            nc.sync.dma_start(out=wup_t[:], in_=wup[:])
            w1_t = cp.tile([N_RADIAL, HIDDEN], f16)
            nc.sync.dma_start(out=w1_t[:], in_=w1d[:])
            w2_t = cp.tile([HIDDEN, HIDDEN], f16)
            nc.sync.dma_start(out=w2_t[:], in_=w2d[:])
            w3_t = cp.tile([HIDDEN, HIDDEN], f16)
            nc.sync.dma_start(out=w3_t[:], in_=w3d[:])
            w4_t = cp.tile([HIDDEN, 512], f16)
            nc.sync.dma_start(out=w4_t[:], in_=w4d[:])
            wlin_t = cp.tile([P, 512], f16)
            nc.sync.dma_start(out=wlin_t[:], in_=wlind[:])
            iota_t = cp.tile([P, P], f32)
            nc.sync.dma_start(out=iota_t[:], in_=iotad[:])
            ident_t = cp.tile([P, P], f16)
            nc.sync.dma_start(out=ident_t[:], in_=identd[:])
            zt = cp.tile([P, P], f16)
            nc.vector.memset(zt[:], 0.0)

            # ---- Phase A: up-projection table (all nodes, replicated) ----
            for nt in range(N_NODE_TILES):
                r0 = nt * P
                ups = psW.tile([P, 512], f32, tag="w512", name="ups")
                for b in range(4):
                    xT = up_sb.tile([P, P], f16, tag="xT")
                    nc.sync.dma_start(
                        out=xT[:], in_=nfT[b * P:(b + 1) * P, r0:r0 + P])
                    nc.tensor.matmul(
                        out=ups[:, b * P:(b + 1) * P], lhsT=xT[:],
                        rhs=wup_t[:, b * P:(b + 1) * P], start=True, stop=True)
                urow = up_sb.tile([P, 512], f16, tag="urow")
                if nt % 2 == 0:
                    nc.vector.tensor_copy(out=urow[:], in_=ups[:])
                else:
                    nc.scalar.copy(out=urow[:], in_=ups[:])
                nc.sync.dma_start(out=table[r0:r0 + P, :], in_=urow[:])

            # ---- Phase B: edge chunks ----
            ci_global = 0
            for t in range(TILES_PER_CORE):
                n_chunks = c_prof[t]
                acc = psA.tile([P, 1024], f32, tag="acc")
                nc.tensor.matmul(out=acc[:, 0:512], lhsT=zt[:], rhs=wup_t[:],
                                 start=True, stop=True, skip_group_check=True)
                nc.tensor.matmul(out=acc[:, 512:1024], lhsT=zt[:], rhs=wup_t[:],
                                 start=True, stop=True, skip_group_check=True)
                for ci in range(n_chunks):
                    e0 = ci_global * P
                    ci_global += 1
                    gidx = ep.tile([P, 1], i32, tag="gidx")
                    nc.sync.dma_start(out=gidx[:], in_=gidxd[e0:e0 + P, :])
                    rloc = ep.tile([P, 1], f32, tag="rloc")
                    nc.sync.dma_start(out=rloc[:], in_=rlocd[e0:e0 + P, :])
                    at = ep.tile([P, 4], f32, tag="at")
                    nc.sync.dma_start(out=at[:], in_=attrd[e0:e0 + P, :])
                    eft = ep.tile([N_RADIAL, P], f16, tag="eft")
                    nc.sync.dma_start(out=eft[:], in_=eftd[:, e0:e0 + P])
                    g = ep.tile([P, 512], f16, tag="g")
                    nc.gpsimd.indirect_dma_start(
                        out=g[:], out_offset=None, in_=table[:],
                        in_offset=bass.IndirectOffsetOnAxis(ap=gidx[:, :1], axis=0))

                    # radial MLP (PE + ACT silu)
                    hps = psH.tile([HIDDEN, 3 * P], f32, tag="hps")
                    nc.tensor.matmul(out=hps[:, 0:P], lhsT=w1_t[:], rhs=eft[:],
                                     start=True, stop=True)
                    h1 = pp.tile([HIDDEN, P], f16, tag="h1")
                    nc.scalar.activation(out=h1[:], in_=hps[:, 0:P], func=SILU)
                    nc.tensor.matmul(out=hps[:, P:2 * P], lhsT=w2_t[:], rhs=h1[:],
                                     start=True, stop=True)
                    h2 = pp.tile([HIDDEN, P], f16, tag="h2")
                    nc.scalar.activation(out=h2[:], in_=hps[:, P:2 * P], func=SILU)
                    nc.tensor.matmul(out=hps[:, 2 * P:3 * P], lhsT=w3_t[:], rhs=h2[:],
                                     start=True, stop=True)
                    h3 = pp.tile([HIDDEN, P], f16, tag="h3")
                    nc.scalar.activation(out=h3[:], in_=hps[:, 2 * P:3 * P], func=SILU)
                    tpw = psW.tile([P, 512], f32, tag="w512", name="tpw")
                    nc.tensor.matmul(out=tpw[:], lhsT=h3[:], rhs=w4_t[:],
                                     start=True, stop=True)
                    wt = pp.tile([P, 512], f16, tag="wt")   # w0|w1|w2|w3
                    if ci % 2 == 0:
                        nc.vector.tensor_copy(out=wt[:], in_=tpw[:])
                    else:
                        nc.scalar.copy(out=wt[:], in_=tpw[:])

                    # per-edge tensor product -> mji [128e, 1024]
                    # layout: [m0a | m0b | m1a(3) | m1b(3)]
                    # gather-only-dependent ops first (keep DVE fed while
                    # the MLP chain produces wt)
                    y0 = at[:, 0:1]
                    oh = pp.tile([P, P], f16, tag="oh")
                    nc.vector.tensor_scalar(
                        out=oh[:], in0=iota_t[:], scalar1=rloc[:, :1], scalar2=None,
                        op0=EQ_)
                    vy = pp.tile([P, 3 * P], f16, tag="vy")
                    nc.vector.tensor_tensor(
                        out=vy[:].rearrange("p (m u) -> p m u", u=P),
                        in0=g[:, P:4 * P].rearrange("p (m u) -> p m u", u=P),
                        in1=at[:, 1:4].to_broadcast([P, 3, P]),
                        op=MUL_)
                    d01 = pp.tile([P, P], f16, tag="d01")
                    nc.vector.tensor_add(out=d01[:], in0=vy[:, 0:P],
                                         in1=vy[:, P:2 * P])
                    d2 = pp.tile([P, P], f16, tag="d2")
                    nc.gpsimd.tensor_add(out=d2[:], in0=d01[:], in1=vy[:, 2 * P:3 * P])
                    mji = pp.tile([P, 1024], f16, tag="mji")
                    nc.vector.scalar_tensor_tensor(
                        out=mji[:, 0:P], in0=g[:, 0:P], scalar=y0, in1=wt[:, 0:P],
                        op0=MUL_, op1=MUL_)
                    sw2 = pp.tile([P, P], f16, tag="sw2")
                    nc.vector.tensor_mul(out=sw2[:], in0=g[:, 0:P],
                                         in1=wt[:, 2 * P:3 * P])
                    for m in range(3):
                        nc.scalar.activation(
                            out=mji[:, (2 + m) * P:(3 + m) * P], in_=sw2[:],
                            func=COPY, scale=at[:, 1 + m:2 + m])
                    w3y0 = pp.tile([P, P], f16, tag="w3y0")
                    nc.vector.tensor_scalar(
                        out=w3y0[:], in0=wt[:, 3 * P:4 * P], scalar1=y0,
                        scalar2=None, op0=MUL_)
                    for m in range(3):
                        nc.vector.tensor_mul(
                            out=mji[:, (5 + m) * P:(6 + m) * P],
                            in0=g[:, (1 + m) * P:(2 + m) * P], in1=w3y0[:])
                    nc.gpsimd.tensor_mul(out=mji[:, P:2 * P], in0=d2[:],
                                         in1=wt[:, P:2 * P])

                    nc.tensor.matmul(out=acc[:, 0:512], lhsT=oh[:],
                                     rhs=mji[:, 0:512], start=False,
                                     stop=(ci == n_chunks - 1),
                                     skip_group_check=True)
                    nc.tensor.matmul(out=acc[:, 512:1024], lhsT=oh[:],
                                     rhs=mji[:, 512:1024], start=False,
                                     stop=(ci == n_chunks - 1),
                                     skip_group_check=True)

                # ---- flush node tile t ----
                msg = fp.tile([P, 1024], f16, tag="msg")
                nc.vector.tensor_copy(out=msg[:, 0:512], in_=acc[:, 0:512])
                nc.scalar.copy(out=msg[:, 512:1024], in_=acc[:, 512:1024])
                psT = psF.tile([P, 1024], f16, tag="psTfin", name="psT")
                for b in range(8):
                    nc.tensor.transpose(
                        out=psT[:, b * P:(b + 1) * P],
                        in_=msg[:, b * P:(b + 1) * P], identity=ident_t[:])
                msgT = fp.tile([P, 1024], f16, tag="msgT")
                nc.vector.tensor_copy(out=msgT[:, 0:512], in_=psT[:, 0:512])
                nc.scalar.copy(out=msgT[:, 512:1024], in_=psT[:, 512:1024])
                fin = psF.tile([P, 512], f32, tag="psTfin", name="fin")
                nc.tensor.matmul(out=fin[:], lhsT=zt[:], rhs=wup_t[:],
                                 start=True, stop=True, skip_group_check=True)
                nc.tensor.matmul(out=fin[:, 0:P], lhsT=msgT[:, 0:P],
                                 rhs=wlin_t[:, 0:P], start=False, stop=False,
                                 skip_group_check=True)
                nc.tensor.matmul(out=fin[:, 0:P], lhsT=msgT[:, P:2 * P],
                                 rhs=wlin_t[:, P:2 * P], start=False, stop=True,
                                 skip_group_check=True)
                for m in range(3):
                    nc.tensor.matmul(
                        out=fin[:, (1 + m) * P:(2 + m) * P],
                        lhsT=msgT[:, (2 + m) * P:(3 + m) * P],
                        rhs=wlin_t[:, 2 * P:3 * P], start=False, stop=False,
                        skip_group_check=True)
                    nc.tensor.matmul(
                        out=fin[:, (1 + m) * P:(2 + m) * P],
                        lhsT=msgT[:, (5 + m) * P:(6 + m) * P],
                        rhs=wlin_t[:, 3 * P:4 * P], start=False, stop=True,
                        skip_group_check=True)
                ot = fp.tile([P, 512], f32, tag="ot")
                nc.vector.tensor_copy(out=ot[:, 0:P], in_=fin[:, 0:P])
                for m in range(3):
                    dst = ot[:, P + m:512:3]
                    if m == 0:
                        nc.vector.tensor_copy(out=dst, in_=fin[:, P:2 * P])
                    elif m == 1:
                        nc.scalar.copy(out=dst, in_=fin[:, 2 * P:3 * P])
                    else:
                        nc.vector.tensor_copy(out=dst, in_=fin[:, 3 * P:4 * P])
                nc.sync.dma_start(out=outd[t * P:(t + 1) * P, :], in_=ot[:])

    nc.compile()
    return nc


def _host_prep(inputs):
    nf = np.asarray(inputs["node_feats"], dtype=np.float32)
    ea = np.asarray(inputs["edge_attrs"], dtype=np.float32)
    ef = np.asarray(inputs["edge_feats"], dtype=np.float32)
    snd = np.asarray(inputs["sender"]).astype(np.int64)
    rcv = np.asarray(inputs["receiver"]).astype(np.int64)

    inv = 1.0 / math.sqrt(MUL)
    inv2 = 1.0 / math.sqrt(2 * MUL)
    c = 1.0 / math.sqrt(MUL)
    c3 = 1.0 / math.sqrt(3.0 * MUL)

    # node feats fp16, transposed block-major: row b*128+ch, col n
    s = nf[:, :MUL]
    v = nf[:, MUL:].reshape(-1, MUL, 3)
    nfT = np.zeros((512, NODE_PAD), np.float16)
    nfT[0:128, :N_NODES] = s.T
    for m in range(3):
        nfT[128 * (1 + m):128 * (2 + m), :N_NODES] = v[:, :, m].T

    wup = np.zeros((P, 512), np.float16)
    wup[:, 0:128] = (np.asarray(inputs["W_up0"]) * inv).astype(np.float16)
    w_up1 = (np.asarray(inputs["W_up1"]) * inv).astype(np.float16)
    for m in range(3):
        wup[:, 128 * (1 + m):128 * (2 + m)] = w_up1
    w1 = (np.asarray(inputs["mlp_w1"]) / math.sqrt(N_RADIAL)).astype(np.float16)
    w2 = (np.asarray(inputs["mlp_w2"]) / math.sqrt(HIDDEN)).astype(np.float16)
    w3 = (np.asarray(inputs["mlp_w3"]) / math.sqrt(HIDDEN)).astype(np.float16)
    w4 = np.asarray(inputs["mlp_w4"]) / math.sqrt(HIDDEN)
    w4 = w4 * np.concatenate([np.full(128, c), np.full(128, c3),
                              np.full(128, c), np.full(128, c)])
    w4 = w4.astype(np.float16)
    wlin = np.zeros((P, 512), np.float16)
    lin0 = (np.asarray(inputs["W_lin0"]) * inv2 / 10.0).astype(np.float16)
    lin1 = (np.asarray(inputs["W_lin1"]) * inv2 / 10.0).astype(np.float16)
    wlin[:, 0:128] = lin0[:128]
    wlin[:, 128:256] = lin0[128:]
    wlin[:, 256:384] = lin1[:128]
    wlin[:, 384:512] = lin1[128:]

    iota = np.tile(np.arange(P, dtype=np.float32), (P, 1))
    ident = np.eye(P, dtype=np.float16)

    core_of = rcv // NODES_PER_CORE
    tile_of = (rcv % NODES_PER_CORE) // P
    sizes = np.zeros((NCORES, TILES_PER_CORE), np.int64)
    np.add.at(sizes, (core_of, tile_of), 1)
    c_prof = tuple(max(1, int(math.ceil(sizes[:, t].max() / P)))
                   for t in range(TILES_PER_CORE))
    nch = sum(c_prof)
    ne_pad = nch * P

    order = np.lexsort((rcv, tile_of, core_of))
    gidx_all = np.zeros((NCORES, ne_pad, 1), np.int32)
    rloc_all = np.zeros((NCORES, ne_pad, 1), np.float32)
    attr_all = np.zeros((NCORES, ne_pad, 4), np.float32)
    eft_all = np.zeros((NCORES, N_RADIAL, ne_pad), np.float16)

    starts = np.concatenate([[0], np.cumsum(np.asarray(c_prof)) * P])[:-1]
    flat_sizes = sizes.reshape(-1)
    run_start = np.concatenate([[0], np.cumsum(flat_sizes)])[:-1].reshape(
        NCORES, TILES_PER_CORE)

    for cidx in range(NCORES):
        for t in range(TILES_PER_CORE):
            n = int(sizes[cidx, t])
            if n == 0:
                continue
            e = order[run_start[cidx, t]:run_start[cidx, t] + n]
            s0 = int(starts[t])
            gidx_all[cidx, s0:s0 + n, 0] = snd[e]
            rloc_all[cidx, s0:s0 + n, 0] = (rcv[e] % NODES_PER_CORE) - t * P
            attr_all[cidx, s0:s0 + n, :] = ea[e]
            eft_all[cidx, :, s0:s0 + n] = ef[e].astype(np.float16).T

    common = dict(nfT=nfT, wup=wup, w1d=w1, w2d=w2, w3d=w3, w4d=w4,
                  wlind=wlin, iotad=iota, identd=ident)
    in_maps = []
    for cidx in range(NCORES):
        m = dict(common)
        m.update(gidxd=gidx_all[cidx], rlocd=rloc_all[cidx],
                 attrd=attr_all[cidx], eftd=eft_all[cidx])
        in_maps.append(m)
    return c_prof, in_maps


def kernel(**inputs):
    from concourse.bass_utils import run_bass_kernel_spmd

    c_prof, in_maps = _host_prep(inputs)
    if c_prof not in _CACHE:
        _CACHE[c_prof] = _build(c_prof)
    nc = _CACHE[c_prof]

    trace = bool(os.environ.get("KERNEL_TRACE"))
    if trace:
        import sys, types
        import concourse.bass_utils as bu
        try:
            import antenv.axon_hooks  # noqa
        except ImportError:
            import trn_agent_boot.trn_boot as tb
            hooks = types.ModuleType("antenv.axon_hooks")
            hk = tb._ntff_profile_via_ctypes("/opt/axon/libaxon_pjrt.so")
            hooks.get_axon_ntff_profile_hook = lambda: hk
            hooks.set_axon_ntff_profile_hook = lambda h: None
            sys.modules["antenv.axon_hooks"] = hooks
        bu.upload_artifacts = lambda d: d

    res = run_bass_kernel_spmd(nc, in_maps, list(range(NCORES)), trace=trace)
    if trace and res.exec_time_ns is not None:
        print(f"HW exec time: {res.exec_time_ns} ns")
        if res.instructions_and_trace:
            print(f"trace: {res.instructions_and_trace[1]}")

    out = np.empty((N_NODES, 512), np.float32)
    for cidx in range(NCORES):
        lo = cidx * NODES_PER_CORE
        hi = min((cidx + 1) * NODES_PER_CORE, N_NODES)
        if lo >= N_NODES:
            break
        out[lo:hi] = res.results[cidx]["outd"][:hi - lo]
    return out


# revision 11
# speedup vs baseline: 1.0250x; 1.0041x over previous
"""E3nn interaction (gnn message passing) Bass kernel for 8 Trainium2 cores.

Strategy: edges are sorted by receiver and partitioned so core i owns the
segment-sum for nodes [2560*i, 2560*(i+1)).  Each core redundantly computes
the up-projected node table (fp16) into its own DRAM from host-pretransposed
features, then streams its edge chunks: indirect-gather of sender rows,
radial MLP on the tensor engine, fused per-edge tensor-product ops on
DVE/ACT/POOL, and a one-hot matmul scatter accumulating messages in PSUM.
Per 128-node tile the accumulator is transposed on PE and the final linear
is applied, writing the core's output rows directly.
"""
import math
import os
import numpy as np

N_NODES = 20000
N_EDGES = 200000
MUL = 128
P = 128
NCORES = 8
TILES_PER_CORE = 20
NODES_PER_CORE = TILES_PER_CORE * P          # 2560
NODE_PAD = NCORES * NODES_PER_CORE           # 20480
N_NODE_TILES = NODE_PAD // P                 # 160
N_RADIAL = 8
HIDDEN = 64

_CACHE = {}


def _build(c_prof):
    import concourse.bacc as bacc
    import concourse.bass as bass
    import concourse.tile as tile
    from concourse import mybir

    f16, f32, i32 = mybir.dt.float16, mybir.dt.float32, mybir.dt.int32
    MUL_ = mybir.AluOpType.mult
    ADD_ = mybir.AluOpType.add
    EQ_ = mybir.AluOpType.is_equal
    SILU = mybir.ActivationFunctionType.Silu
    COPY = mybir.ActivationFunctionType.Copy

    nch = sum(c_prof)
    ne_pad = nch * P

    nc = bacc.Bacc()
    nfT = nc.declare_dram_parameter("nfT", [512, NODE_PAD], f16, isOutput=False)
    wup = nc.declare_dram_parameter("wup", [P, 512], f16, isOutput=False)
    w1d = nc.declare_dram_parameter("w1d", [N_RADIAL, HIDDEN], f16, isOutput=False)
    w2d = nc.declare_dram_parameter("w2d", [HIDDEN, HIDDEN], f16, isOutput=False)
    w3d = nc.declare_dram_parameter("w3d", [HIDDEN, HIDDEN], f16, isOutput=False)
    w4d = nc.declare_dram_parameter("w4d", [HIDDEN, 512], f16, isOutput=False)
    wlind = nc.declare_dram_parameter("wlind", [P, 512], f16, isOutput=False)
    iotad = nc.declare_dram_parameter("iotad", [P, P], f32, isOutput=False)
    identd = nc.declare_dram_parameter("identd", [P, P], f16, isOutput=False)
    gidxd = nc.declare_dram_parameter("gidxd", [ne_pad, 1], i32, isOutput=False)
    rlocd = nc.declare_dram_parameter("rlocd", [ne_pad, 1], f32, isOutput=False)
    attrd = nc.declare_dram_parameter("attrd", [ne_pad, 4], f32, isOutput=False)
    eftd = nc.declare_dram_parameter("eftd", [N_RADIAL, ne_pad], f16, isOutput=False)
    outd = nc.declare_dram_parameter("outd", [NODES_PER_CORE, 512], f32, isOutput=True)

    with tile.TileContext(nc) as tc:
        with tc.tile_pool(name="const", bufs=1) as cp, \
             tc.tile_pool(name="dram", bufs=1, space="DRAM") as dp, \
             tc.tile_pool(name="upsb", bufs=10) as up_sb, \
             tc.tile_pool(name="edge", bufs=8) as ep, \
             tc.tile_pool(name="prod", bufs=6) as pp, \
             tc.tile_pool(name="flush", bufs=3) as fp, \
             tc.tile_pool(name="psA", bufs=1, space="PSUM") as psA, \
             tc.tile_pool(name="psW", bufs=3, space="PSUM") as psW, \
             tc.tile_pool(name="psH", bufs=2, space="PSUM") as psH, \
             tc.tile_pool(name="psF", bufs=1, space="PSUM") as psF:

            table = dp.tile([NODE_PAD, 512], f16)

            wup_t = cp.tile([P, 512], f16)
            nc.sync.dma_start(out=wup_t[:], in_=wup[:])
            w1_t = cp.tile([N_RADIAL, HIDDEN], f16)
            nc.sync.dma_start(out=w1_t[:], in_=w1d[:])
            w2_t = cp.tile([HIDDEN, HIDDEN], f16)
            nc.sync.dma_start(out=w2_t[:], in_=w2d[:])
            w3_t = cp.tile([HIDDEN, HIDDEN], f16)
            nc.sync.dma_start(out=w3_t[:], in_=w3d[:])
            w4_t = cp.tile([HIDDEN, 512], f16)
            nc.sync.dma_start(out=w4_t[:], in_=w4d[:])
            wlin_t = cp.tile([P, 512], f16)
            nc.sync.dma_start(out=wlin_t[:], in_=wlind[:])
            iota_t = cp.tile([P, P], f32)
            nc.sync.dma_start(out=iota_t[:], in_=iotad[:])
            ident_t = cp.tile([P, P], f16)
            nc.sync.dma_start(out=ident_t[:], in_=identd[:])
            zt = cp.tile([P, P], f16)
            nc.vector.memset(zt[:], 0.0)

            # ---- Phase A: up-projection table (all nodes, replicated) ----
            for nt in range(N_NODE_TILES):
                r0 = nt * P
                ups = psW.tile([P, 512], f32, tag="w512", name="ups")
                for b in range(4):
                    xT = up_sb.tile([P, P], f16, tag="xT")
                    nc.sync.dma_start(
                        out=xT[:], in_=nfT[b * P:(b + 1) * P, r0:r0 + P])
                    nc.tensor.matmul(
                        out=ups[:, b * P:(b + 1) * P], lhsT=xT[:],
                        rhs=wup_t[:, b * P:(b + 1) * P], start=True, stop=True)
                urow = up_sb.tile([P, 512], f16, tag="urow")
                if nt % 2 == 0:
                    nc.vector.tensor_copy(out=urow[:], in_=ups[:])
                else:
                    nc.scalar.copy(out=urow[:], in_=ups[:])
                nc.sync.dma_start(out=table[r0:r0 + P, :], in_=urow[:])

            # ---- Phase B: edge chunks ----
            ci_global = 0
            for t in range(TILES_PER_CORE):
                n_chunks = c_prof[t]
                acc = psA.tile([P, 1024], f32, tag="acc")
                nc.tensor.matmul(out=acc[:, 0:512], lhsT=zt[:], rhs=wup_t[:],
                                 start=True, stop=True, skip_group_check=True)
                nc.tensor.matmul(out=acc[:, 512:1024], lhsT=zt[:], rhs=wup_t[:],
                                 start=True, stop=True, skip_group_check=True)
                for ci in range(n_chunks):
                    e0 = ci_global * P
                    ci_global += 1
                    gidx = ep.tile([P, 1], i32, tag="gidx")
                    nc.sync.dma_start(out=gidx[:], in_=gidxd[e0:e0 + P, :])
                    rloc = ep.tile([P, 1], f32, tag="rloc")
                    nc.sync.dma_start(out=rloc[:], in_=rlocd[e0:e0 + P, :])
                    at = ep.tile([P, 4], f32, tag="at")
                    nc.sync.dma_start(out=at[:], in_=attrd[e0:e0 + P, :])
                    eft = ep.tile([N_RADIAL, P], f16, tag="eft")
                    nc.sync.dma_start(out=eft[:], in_=eftd[:, e0:e0 + P])
                    g = ep.tile([P, 512], f16, tag="g")
                    nc.gpsimd.indirect_dma_start(
                        out=g[:], out_offset=None, in_=table[:],
                        in_offset=bass.IndirectOffsetOnAxis(ap=gidx[:, :1], axis=0))

                    # radial MLP (PE + ACT silu)
                    hps = psH.tile([HIDDEN, 3 * P], f32, tag="hps")
                    nc.tensor.matmul(out=hps[:, 0:P], lhsT=w1_t[:], rhs=eft[:],
                                     start=True, stop=True)
                    h1 = pp.tile([HIDDEN, P], f16, tag="h1")
                    nc.scalar.activation(out=h1[:], in_=hps[:, 0:P], func=SILU)
                    nc.tensor.matmul(out=hps[:, P:2 * P], lhsT=w2_t[:], rhs=h1[:],
                                     start=True, stop=True)
                    h2 = pp.tile([HIDDEN, P], f16, tag="h2")
                    nc.scalar.activation(out=h2[:], in_=hps[:, P:2 * P], func=SILU)
                    nc.tensor.matmul(out=hps[:, 2 * P:3 * P], lhsT=w3_t[:], rhs=h2[:],
                                     start=True, stop=True)
                    h3 = pp.tile([HIDDEN, P], f16, tag="h3")
                    nc.scalar.activation(out=h3[:], in_=hps[:, 2 * P:3 * P], func=SILU)
                    tpw = psW.tile([P, 512], f32, tag="w512", name="tpw")
                    nc.tensor.matmul(out=tpw[:], lhsT=h3[:], rhs=w4_t[:],
                                     start=True, stop=True)
                    wt = pp.tile([P, 512], f16, tag="wt")   # w0|w1|w2|w3
                    if ci % 2 == 0:
                        nc.vector.tensor_copy(out=wt[:], in_=tpw[:])
                    else:
                        nc.scalar.copy(out=wt[:], in_=tpw[:])

                    # per-edge tensor product -> mji [128e, 1024]
                    # layout: [m0a | m0b | m1a(3) | m1b(3)]
                    # gather-only-dependent ops first (keep DVE fed while
                    # the MLP chain produces wt)
                    y0 = at[:, 0:1]
                    oh = pp.tile([P, P], f16, tag="oh")
                    nc.vector.tensor_scalar(
                        out=oh[:], in0=iota_t[:], scalar1=rloc[:, :1], scalar2=None,
                        op0=EQ_)
                    vy = pp.tile([P, 3 * P], f16, tag="vy")
                    nc.vector.tensor_tensor(
                        out=vy[:].rearrange("p (m u) -> p m u", u=P),
                        in0=g[:, P:4 * P].rearrange("p (m u) -> p m u", u=P),
                        in1=at[:, 1:4].to_broadcast([P, 3, P]),
                        op=MUL_)
                    d01 = pp.tile([P, P], f16, tag="d01")
                    nc.vector.tensor_add(out=d01[:], in0=vy[:, 0:P],
                                         in1=vy[:, P:2 * P])
                    d2 = pp.tile([P, P], f16, tag="d2")
                    nc.gpsimd.tensor_add(out=d2[:], in0=d01[:], in1=vy[:, 2 * P:3 * P])
                    mji = pp.tile([P, 1024], f16, tag="mji")
                    nc.vector.scalar_tensor_tensor(
                        out=mji[:, 0:P], in0=g[:, 0:P], scalar=y0, in1=wt[:, 0:P],
                        op0=MUL_, op1=MUL_)
                    sw2 = pp.tile([P, P], f16, tag="sw2")
                    nc.vector.tensor_mul(out=sw2[:], in0=g[:, 0:P],
                                         in1=wt[:, 2 * P:3 * P])
                    for m in range(3):
                        nc.scalar.activation(
                            out=mji[:, (2 + m) * P:(3 + m) * P], in_=sw2[:],
                            func=COPY, scale=at[:, 1 + m:2 + m])
                    w3y0 = pp.tile([P, P], f16, tag="w3y0")
                    nc.vector.tensor_scalar(
                        out=w3y0[:], in0=wt[:, 3 * P:4 * P], scalar1=y0,
                        scalar2=None, op0=MUL_)
                    for m in range(3):
                        nc.vector.tensor_mul(
                            out=mji[:, (5 + m) * P:(6 + m) * P],
                            in0=g[:, (1 + m) * P:(2 + m) * P], in1=w3y0[:])
                    nc.gpsimd.tensor_mul(out=mji[:, P:2 * P], in0=d2[:],
                                         in1=wt[:, P:2 * P])

                    nc.tensor.matmul(out=acc[:, 0:512], lhsT=oh[:],
                                     rhs=mji[:, 0:512], start=False,
                                     stop=(ci == n_chunks - 1),
                                     skip_group_check=True)
                    nc.tensor.matmul(out=acc[:, 512:1024], lhsT=oh[:],
                                     rhs=mji[:, 512:1024], start=False,
                                     stop=(ci == n_chunks - 1),
                                     skip_group_check=True)

                # ---- flush node tile t ----
                msg = fp.tile([P, 1024], f16, tag="msg")
                nc.vector.tensor_copy(out=msg[:, 0:512], in_=acc[:, 0:512])
                nc.scalar.copy(out=msg[:, 512:1024], in_=acc[:, 512:1024])
                psT = psF.tile([P, 1024], f16, tag="psTfin", name="psT")
                for b in range(8):
                    nc.tensor.transpose(
                        out=psT[:, b * P:(b + 1) * P],
                        in_=msg[:, b * P:(b + 1) * P], identity=ident_t[:])
                msgT = fp.tile([P, 1024], f16, tag="msgT")
                nc.vector.tensor_copy(out=msgT[:, 0:512], in_=psT[:, 0:512])
                nc.scalar.copy(out=msgT[:, 512:1024], in_=psT[:, 512:1024])
                fin = psF.tile([P, 512], f32, tag="psTfin", name="fin")
                nc.tensor.matmul(out=fin[:], lhsT=zt[:], rhs=wup_t[:],
                                 start=True, stop=True, skip_group_check=True)
                nc.tensor.matmul(out=fin[:, 0:P], lhsT=msgT[:, 0:P],
                                 rhs=wlin_t[:, 0:P], start=False, stop=False,
                                 skip_group_check=True)
                nc.tensor.matmul(out=fin[:, 0:P], lhsT=msgT[:, P:2 * P],
                                 rhs=wlin_t[:, P:2 * P], start=False, stop=True,
                                 skip_group_check=True)
                for m in range(3):
                    nc.tensor.matmul(
                        out=fin[:, (1 + m) * P:(2 + m) * P],
                        lhsT=msgT[:, (2 + m) * P:(3 + m) * P],
                        rhs=wlin_t[:, 2 * P:3 * P], start=False, stop=False,
                        skip_group_check=True)
                    nc.tensor.matmul(
                        out=fin[:, (1 + m) * P:(2 + m) * P],
                        lhsT=msgT[:, (5 + m) * P:(6 + m) * P],
                        rhs=wlin_t[:, 3 * P:4 * P], start=False, stop=True,
                        skip_group_check=True)
                ot = fp.tile([P, 512], f32, tag="ot")
                nc.vector.tensor_copy(out=ot[:, 0:P], in_=fin[:, 0:P])
                for m in range(3):
                    dst = ot[:, P + m:512:3]
                    if m == 0:
                        nc.vector.tensor_copy(out=dst, in_=fin[:, P:2 * P])
                    elif m == 1:
                        nc.scalar.copy(out=dst, in_=fin[:, 2 * P:3 * P])
                    else:
                        nc.vector.tensor_copy(out=dst, in_=fin[:, 3 * P:4 * P])
                nc.sync.dma_start(out=outd[t * P:(t + 1) * P, :], in_=ot[:])

    nc.compile()
    return nc


def _host_prep(inputs):
    nf = np.asarray(inputs["node_feats"], dtype=np.float32)
    ea = np.asarray(inputs["edge_attrs"], dtype=np.float32)
    ef = np.asarray(inputs["edge_feats"], dtype=np.float32)
    snd = np.asarray(inputs["sender"]).astype(np.int64)
    rcv = np.asarray(inputs["receiver"]).astype(np.int64)

    inv = 1.0 / math.sqrt(MUL)
    inv2 = 1.0 / math.sqrt(2 * MUL)
    c = 1.0 / math.sqrt(MUL)
    c3 = 1.0 / math.sqrt(3.0 * MUL)

    # node feats fp16, transposed block-major: row b*128+ch, col n
    s = nf[:, :MUL]
    v = nf[:, MUL:].reshape(-1, MUL, 3)
    nfT = np.zeros((512, NODE_PAD), np.float16)
    nfT[0:128, :N_NODES] = s.T
    for m in range(3):
        nfT[128 * (1 + m):128 * (2 + m), :N_NODES] = v[:, :, m].T

    wup = np.zeros((P, 512), np.float16)
    wup[:, 0:128] = (np.asarray(inputs["W_up0"]) * inv).astype(np.float16)
    w_up1 = (np.asarray(inputs["W_up1"]) * inv).astype(np.float16)
    for m in range(3):
        wup[:, 128 * (1 + m):128 * (2 + m)] = w_up1
    w1 = (np.asarray(inputs["mlp_w1"]) / math.sqrt(N_RADIAL)).astype(np.float16)
    w2 = (np.asarray(inputs["mlp_w2"]) / math.sqrt(HIDDEN)).astype(np.float16)
    w3 = (np.asarray(inputs["mlp_w3"]) / math.sqrt(HIDDEN)).astype(np.float16)
    w4 = np.asarray(inputs["mlp_w4"]) / math.sqrt(HIDDEN)
    w4 = w4 * np.concatenate([np.full(128, c), np.full(128, c3),
                              np.full(128, c), np.full(128, c)])
    w4 = w4.astype(np.float16)
    wlin = np.zeros((P, 512), np.float16)
    lin0 = (np.asarray(inputs["W_lin0"]) * inv2 / 10.0).astype(np.float16)
    lin1 = (np.asarray(inputs["W_lin1"]) * inv2 / 10.0).astype(np.float16)
    wlin[:, 0:128] = lin0[:128]
    wlin[:, 128:256] = lin0[128:]
    wlin[:, 256:384] = lin1[:128]
    wlin[:, 384:512] = lin1[128:]

    iota = np.tile(np.arange(P, dtype=np.float32), (P, 1))
    ident = np.eye(P, dtype=np.float16)

    core_of = rcv // NODES_PER_CORE
    tile_of = (rcv % NODES_PER_CORE) // P
    sizes = np.zeros((NCORES, TILES_PER_CORE), np.int64)
    np.add.at(sizes, (core_of, tile_of), 1)
    c_prof = tuple(max(1, int(math.ceil(sizes[:, t].max() / P)))
                   for t in range(TILES_PER_CORE))
    nch = sum(c_prof)
    ne_pad = nch * P

    order = np.lexsort((rcv, tile_of, core_of))
    gidx_all = np.zeros((NCORES, ne_pad, 1), np.int32)
    rloc_all = np.zeros((NCORES, ne_pad, 1), np.float32)
    attr_all = np.zeros((NCORES, ne_pad, 4), np.float32)
    eft_all = np.zeros((NCORES, N_RADIAL, ne_pad), np.float16)

    starts = np.concatenate([[0], np.cumsum(np.asarray(c_prof)) * P])[:-1]
    flat_sizes = sizes.reshape(-1)
    run_start = np.concatenate([[0], np.cumsum(flat_sizes)])[:-1].reshape(
        NCORES, TILES_PER_CORE)

    for cidx in range(NCORES):
        for t in range(TILES_PER_CORE):
            n = int(sizes[cidx, t])
            if n == 0:
                continue
            e = order[run_start[cidx, t]:run_start[cidx, t] + n]
            s0 = int(starts[t])
            gidx_all[cidx, s0:s0 + n, 0] = snd[e]
            rloc_all[cidx, s0:s0 + n, 0] = (rcv[e] % NODES_PER_CORE) - t * P
            attr_all[cidx, s0:s0 + n, :] = ea[e]
            eft_all[cidx, :, s0:s0 + n] = ef[e].astype(np.float16).T

    common = dict(nfT=nfT, wup=wup, w1d=w1, w2d=w2, w3d=w3, w4d=w4,
                  wlind=wlin, iotad=iota, identd=ident)
    in_maps = []
    for cidx in range(NCORES):
        m = dict(common)
        m.update(gidxd=gidx_all[cidx], rlocd=rloc_all[cidx],
                 attrd=attr_all[cidx], eftd=eft_all[cidx])
        in_maps.append(m)
    return c_prof, in_maps


def kernel(**inputs):
    from concourse.bass_utils import run_bass_kernel_spmd

    c_prof, in_maps = _host_prep(inputs)
    if c_prof not in _CACHE:
        _CACHE[c_prof] = _build(c_prof)
    nc = _CACHE[c_prof]

    trace = bool(os.environ.get("KERNEL_TRACE"))
    if trace:
        import sys, types
        import concourse.bass_utils as bu
        try:
            import antenv.axon_hooks  # noqa
        except ImportError:
            import trn_agent_boot.trn_boot as tb
            hooks = types.ModuleType("antenv.axon_hooks")
            hk = tb._ntff_profile_via_ctypes("/opt/axon/libaxon_pjrt.so")
            hooks.get_axon_ntff_profile_hook = lambda: hk
            hooks.set_axon_ntff_profile_hook = lambda h: None
            sys.modules["antenv.axon_hooks"] = hooks
        bu.upload_artifacts = lambda d: d

    res = run_bass_kernel_spmd(nc, in_maps, list(range(NCORES)), trace=trace)
    if trace and res.exec_time_ns is not None:
        print(f"HW exec time: {res.exec_time_ns} ns")
        if res.instructions_and_trace:
            print(f"trace: {res.instructions_and_trace[1]}")

    out = np.empty((N_NODES, 512), np.float32)
    for cidx in range(NCORES):
        lo = cidx * NODES_PER_CORE
        hi = min((cidx + 1) * NODES_PER_CORE, N_NODES)
        if lo >= N_NODES:
            break
        out[lo:hi] = res.results[cidx]["outd"][:hi - lo]
    return out


# revision 12
# speedup vs baseline: 1.2704x; 1.2394x over previous
"""E3nn interaction (gnn message passing) Bass kernel for 8 Trainium2 cores.

Strategy: edges are sorted by receiver and partitioned so core i owns the
segment-sum for nodes [2560*i, 2560*(i+1)).  Each core redundantly computes
the up-projected node table (fp16) into its own DRAM from host-pretransposed
features, then streams its edge chunks: indirect-gather of sender rows,
radial MLP on the tensor engine, fused per-edge tensor-product ops on
DVE/ACT/POOL, and a one-hot matmul scatter accumulating messages in PSUM.
Per 128-node tile the accumulator is transposed on PE and the final linear
is applied, writing the core's output rows directly.
"""
import math
import os
import numpy as np

N_NODES = 20000
N_EDGES = 200000
MUL = 128
P = 128
NCORES = 8
TILES_PER_CORE = 20
NODES_PER_CORE = TILES_PER_CORE * P          # 2560
NODE_PAD = NCORES * NODES_PER_CORE           # 20480
N_NODE_TILES = NODE_PAD // P                 # 160
N_RADIAL = 8
HIDDEN = 64

_CACHE = {}


def _build(c_prof):
    import concourse.bacc as bacc
    import concourse.bass as bass
    import concourse.tile as tile
    from concourse import mybir

    f16, f32, i32 = mybir.dt.float16, mybir.dt.float32, mybir.dt.int32
    MUL_ = mybir.AluOpType.mult
    ADD_ = mybir.AluOpType.add
    EQ_ = mybir.AluOpType.is_equal
    SILU = mybir.ActivationFunctionType.Silu
    COPY = mybir.ActivationFunctionType.Copy

    nch = sum(c_prof)
    ne_pad = nch * P

    nc = bacc.Bacc()
    nfT = nc.declare_dram_parameter("nfT", [512, NODE_PAD], f16, isOutput=False)
    wup = nc.declare_dram_parameter("wup", [P, 512], f16, isOutput=False)
    w1d = nc.declare_dram_parameter("w1d", [N_RADIAL, HIDDEN], f16, isOutput=False)
    w2d = nc.declare_dram_parameter("w2d", [HIDDEN, HIDDEN], f16, isOutput=False)
    w3d = nc.declare_dram_parameter("w3d", [HIDDEN, HIDDEN], f16, isOutput=False)
    w4d = nc.declare_dram_parameter("w4d", [HIDDEN, 512], f16, isOutput=False)
    wlind = nc.declare_dram_parameter("wlind", [P, 512], f16, isOutput=False)
    iotad = nc.declare_dram_parameter("iotad", [P, P], f32, isOutput=False)
    identd = nc.declare_dram_parameter("identd", [P, P], f16, isOutput=False)
    erd = nc.declare_dram_parameter("erd", [ne_pad, 6], f32, isOutput=False)
    eftd = nc.declare_dram_parameter("eftd", [N_RADIAL, ne_pad], f16, isOutput=False)
    outd = nc.declare_dram_parameter("outd", [NODES_PER_CORE, 512], f32, isOutput=True)

    with tile.TileContext(nc) as tc:
        with tc.tile_pool(name="const", bufs=1) as cp, \
             tc.tile_pool(name="dram", bufs=1, space="DRAM") as dp, \
             tc.tile_pool(name="upsb", bufs=10) as up_sb, \
             tc.tile_pool(name="edge", bufs=8) as ep, \
             tc.tile_pool(name="prod", bufs=6) as pp, \
             tc.tile_pool(name="flush", bufs=3) as fp, \
             tc.tile_pool(name="psA", bufs=1, space="PSUM") as psA, \
             tc.tile_pool(name="psW", bufs=3, space="PSUM") as psW, \
             tc.tile_pool(name="psH", bufs=2, space="PSUM") as psH, \
             tc.tile_pool(name="psF", bufs=1, space="PSUM") as psF:

            table = dp.tile([NODE_PAD, 512], f16)

            wup_t = cp.tile([P, 512], f16)
            nc.sync.dma_start(out=wup_t[:], in_=wup[:])
            w1_t = cp.tile([N_RADIAL, HIDDEN], f16)
            nc.sync.dma_start(out=w1_t[:], in_=w1d[:])
            w2_t = cp.tile([HIDDEN, HIDDEN], f16)
            nc.sync.dma_start(out=w2_t[:], in_=w2d[:])
            w3_t = cp.tile([HIDDEN, HIDDEN], f16)
            nc.sync.dma_start(out=w3_t[:], in_=w3d[:])
            w4_t = cp.tile([HIDDEN, 512], f16)
            nc.sync.dma_start(out=w4_t[:], in_=w4d[:])
            wlin_t = cp.tile([P, 512], f16)
            nc.sync.dma_start(out=wlin_t[:], in_=wlind[:])
            iota_t = cp.tile([P, P], f32)
            nc.sync.dma_start(out=iota_t[:], in_=iotad[:])
            ident_t = cp.tile([P, P], f16)
            nc.sync.dma_start(out=ident_t[:], in_=identd[:])
            zt = cp.tile([P, P], f16)
            nc.vector.memset(zt[:], 0.0)

            # ---- Phase A: up-projection table (all nodes, replicated) ----
            for nt in range(N_NODE_TILES):
                r0 = nt * P
                ups = psW.tile([P, 512], f32, tag="w512", name="ups")
                xT4 = up_sb.tile([P, 4 * P], f16, tag="xT4")
                nc.sync.dma_start(
                    out=xT4[:].rearrange("p (b n) -> p b n", n=P),
                    in_=nfT[:, r0:r0 + P].rearrange("(b p) n -> p b n", p=P))
                for b in range(4):
                    nc.tensor.matmul(
                        out=ups[:, b * P:(b + 1) * P],
                        lhsT=xT4[:, b * P:(b + 1) * P],
                        rhs=wup_t[:, b * P:(b + 1) * P], start=True, stop=True)
                urow = up_sb.tile([P, 512], f16, tag="urow")
                if nt % 2 == 0:
                    nc.vector.tensor_copy(out=urow[:], in_=ups[:])
                else:
                    nc.scalar.copy(out=urow[:], in_=ups[:])
                nc.sync.dma_start(out=table[r0:r0 + P, :], in_=urow[:])

            # ---- Phase B: edge chunks ----
            ci_global = 0
            for t in range(TILES_PER_CORE):
                n_chunks = c_prof[t]
                acc = psA.tile([P, 1024], f32, tag="acc")
                nc.tensor.matmul(out=acc[:, 0:512], lhsT=zt[:], rhs=wup_t[:],
                                 start=True, stop=True, skip_group_check=True)
                nc.tensor.matmul(out=acc[:, 512:1024], lhsT=zt[:], rhs=wup_t[:],
                                 start=True, stop=True, skip_group_check=True)
                for ci in range(n_chunks):
                    e0 = ci_global * P
                    ci_global += 1
                    k = ci % 2
                    if k == 0:
                        npair = min(2, n_chunks - ci)
                        er2 = ep.tile([P, 12], f32, tag="er2")
                        nc.sync.dma_start(
                            out=er2[:, 0:6 * npair].rearrange(
                                "p (k c) -> p k c", c=6),
                            in_=erd[e0:e0 + npair * P, :].rearrange(
                                "(k p) c -> p k c", p=P))
                        eft2 = ep.tile([N_RADIAL, 2 * P], f16, tag="eft2")
                        nc.sync.dma_start(out=eft2[:, 0:npair * P],
                                          in_=eftd[:, e0:e0 + npair * P])
                    gidx = er2[:, 6 * k:6 * k + 1].bitcast(i32)
                    rloc = er2[:, 6 * k + 1:6 * k + 2]
                    at = er2[:, 6 * k + 2:6 * k + 6]
                    eft = eft2[:, k * P:(k + 1) * P]
                    g = ep.tile([P, 512], f16, tag="g")
                    nc.gpsimd.indirect_dma_start(
                        out=g[:], out_offset=None, in_=table[:],
                        in_offset=bass.IndirectOffsetOnAxis(ap=gidx, axis=0))

                    # radial MLP (PE + ACT silu)
                    hps = psH.tile([HIDDEN, 3 * P], f32, tag="hps")
                    nc.tensor.matmul(out=hps[:, 0:P], lhsT=w1_t[:], rhs=eft,
                                     start=True, stop=True)
                    h1 = pp.tile([HIDDEN, P], f16, tag="h1")
                    nc.scalar.activation(out=h1[:], in_=hps[:, 0:P], func=SILU)
                    nc.tensor.matmul(out=hps[:, P:2 * P], lhsT=w2_t[:], rhs=h1[:],
                                     start=True, stop=True)
                    h2 = pp.tile([HIDDEN, P], f16, tag="h2")
                    nc.scalar.activation(out=h2[:], in_=hps[:, P:2 * P], func=SILU)
                    nc.tensor.matmul(out=hps[:, 2 * P:3 * P], lhsT=w3_t[:], rhs=h2[:],
                                     start=True, stop=True)
                    h3 = pp.tile([HIDDEN, P], f16, tag="h3")
                    nc.scalar.activation(out=h3[:], in_=hps[:, 2 * P:3 * P], func=SILU)
                    tpw = psW.tile([P, 512], f32, tag="w512", name="tpw")
                    nc.tensor.matmul(out=tpw[:], lhsT=h3[:], rhs=w4_t[:],
                                     start=True, stop=True)
                    wt = pp.tile([P, 512], f16, tag="wt")   # w0|w1|w2|w3
                    if ci % 2 == 0:
                        nc.vector.tensor_copy(out=wt[:], in_=tpw[:])
                    else:
                        nc.scalar.copy(out=wt[:], in_=tpw[:])

                    # per-edge tensor product -> mji [128e, 1024]
                    # layout: [m0a | m0b | m1a(3) | m1b(3)]
                    # gather-only-dependent ops first (keep DVE fed while
                    # the MLP chain produces wt)
                    y0 = at[:, 0:1]
                    oh = pp.tile([P, P], f16, tag="oh")
                    nc.vector.tensor_scalar(
                        out=oh[:], in0=iota_t[:], scalar1=rloc, scalar2=None,
                        op0=EQ_)
                    vy = pp.tile([P, 3 * P], f16, tag="vy")
                    nc.vector.tensor_tensor(
                        out=vy[:].rearrange("p (m u) -> p m u", u=P),
                        in0=g[:, P:4 * P].rearrange("p (m u) -> p m u", u=P),
                        in1=at[:, 1:4].to_broadcast([P, 3, P]),
                        op=MUL_)
                    d01 = pp.tile([P, P], f16, tag="d01")
                    nc.vector.tensor_add(out=d01[:], in0=vy[:, 0:P],
                                         in1=vy[:, P:2 * P])
                    d2 = pp.tile([P, P], f16, tag="d2")
                    nc.gpsimd.tensor_add(out=d2[:], in0=d01[:], in1=vy[:, 2 * P:3 * P])
                    mji = pp.tile([P, 1024], f16, tag="mji")
                    nc.vector.scalar_tensor_tensor(
                        out=mji[:, 0:P], in0=g[:, 0:P], scalar=y0, in1=wt[:, 0:P],
                        op0=MUL_, op1=MUL_)
                    sw2 = pp.tile([P, P], f16, tag="sw2")
                    nc.vector.tensor_mul(out=sw2[:], in0=g[:, 0:P],
                                         in1=wt[:, 2 * P:3 * P])
                    for m in range(3):
                        nc.scalar.activation(
                            out=mji[:, (2 + m) * P:(3 + m) * P], in_=sw2[:],
                            func=COPY, scale=at[:, 1 + m:2 + m])
                    w3y0 = pp.tile([P, P], f16, tag="w3y0")
                    nc.vector.tensor_scalar(
                        out=w3y0[:], in0=wt[:, 3 * P:4 * P], scalar1=y0,
                        scalar2=None, op0=MUL_)
                    for m in range(3):
                        nc.vector.tensor_mul(
                            out=mji[:, (5 + m) * P:(6 + m) * P],
                            in0=g[:, (1 + m) * P:(2 + m) * P], in1=w3y0[:])
                    nc.gpsimd.tensor_mul(out=mji[:, P:2 * P], in0=d2[:],
                                         in1=wt[:, P:2 * P])

                    nc.tensor.matmul(out=acc[:, 0:512], lhsT=oh[:],
                                     rhs=mji[:, 0:512], start=False,
                                     stop=(ci == n_chunks - 1),
                                     skip_group_check=True)
                    nc.tensor.matmul(out=acc[:, 512:1024], lhsT=oh[:],
                                     rhs=mji[:, 512:1024], start=False,
                                     stop=(ci == n_chunks - 1),
                                     skip_group_check=True)

                # ---- flush node tile t ----
                msg = fp.tile([P, 1024], f16, tag="msg")
                nc.vector.tensor_copy(out=msg[:, 0:512], in_=acc[:, 0:512])
                nc.scalar.copy(out=msg[:, 512:1024], in_=acc[:, 512:1024])
                psT = psF.tile([P, 1024], f16, tag="psTfin", name="psT")
                for b in range(8):
                    nc.tensor.transpose(
                        out=psT[:, b * P:(b + 1) * P],
                        in_=msg[:, b * P:(b + 1) * P], identity=ident_t[:])
                msgT = fp.tile([P, 1024], f16, tag="msgT")
                nc.vector.tensor_copy(out=msgT[:, 0:512], in_=psT[:, 0:512])
                nc.scalar.copy(out=msgT[:, 512:1024], in_=psT[:, 512:1024])
                fin = psF.tile([P, 512], f32, tag="psTfin", name="fin")
                nc.tensor.matmul(out=fin[:], lhsT=zt[:], rhs=wup_t[:],
                                 start=True, stop=True, skip_group_check=True)
                nc.tensor.matmul(out=fin[:, 0:P], lhsT=msgT[:, 0:P],
                                 rhs=wlin_t[:, 0:P], start=False, stop=False,
                                 skip_group_check=True)
                nc.tensor.matmul(out=fin[:, 0:P], lhsT=msgT[:, P:2 * P],
                                 rhs=wlin_t[:, P:2 * P], start=False, stop=True,
                                 skip_group_check=True)
                for m in range(3):
                    nc.tensor.matmul(
                        out=fin[:, (1 + m) * P:(2 + m) * P],
                        lhsT=msgT[:, (2 + m) * P:(3 + m) * P],
                        rhs=wlin_t[:, 2 * P:3 * P], start=False, stop=False,
                        skip_group_check=True)
                    nc.tensor.matmul(
                        out=fin[:, (1 + m) * P:(2 + m) * P],
                        lhsT=msgT[:, (5 + m) * P:(6 + m) * P],
                        rhs=wlin_t[:, 3 * P:4 * P], start=False, stop=True,
                        skip_group_check=True)
                ot = fp.tile([P, 512], f32, tag="ot")
                nc.vector.tensor_copy(out=ot[:, 0:P], in_=fin[:, 0:P])
                for m in range(3):
                    dst = ot[:, P + m:512:3]
                    if m == 0:
                        nc.vector.tensor_copy(out=dst, in_=fin[:, P:2 * P])
                    elif m == 1:
                        nc.scalar.copy(out=dst, in_=fin[:, 2 * P:3 * P])
                    else:
                        nc.vector.tensor_copy(out=dst, in_=fin[:, 3 * P:4 * P])
                nc.sync.dma_start(out=outd[t * P:(t + 1) * P, :], in_=ot[:])

    nc.compile()
    return nc


def _host_prep(inputs):
    nf = np.asarray(inputs["node_feats"], dtype=np.float32)
    ea = np.asarray(inputs["edge_attrs"], dtype=np.float32)
    ef = np.asarray(inputs["edge_feats"], dtype=np.float32)
    snd = np.asarray(inputs["sender"]).astype(np.int64)
    rcv = np.asarray(inputs["receiver"]).astype(np.int64)

    inv = 1.0 / math.sqrt(MUL)
    inv2 = 1.0 / math.sqrt(2 * MUL)
    c = 1.0 / math.sqrt(MUL)
    c3 = 1.0 / math.sqrt(3.0 * MUL)

    # node feats fp16, transposed block-major: row b*128+ch, col n
    s = nf[:, :MUL]
    v = nf[:, MUL:].reshape(-1, MUL, 3)
    nfT = np.zeros((512, NODE_PAD), np.float16)
    nfT[0:128, :N_NODES] = s.T
    for m in range(3):
        nfT[128 * (1 + m):128 * (2 + m), :N_NODES] = v[:, :, m].T

    wup = np.zeros((P, 512), np.float16)
    wup[:, 0:128] = (np.asarray(inputs["W_up0"]) * inv).astype(np.float16)
    w_up1 = (np.asarray(inputs["W_up1"]) * inv).astype(np.float16)
    for m in range(3):
        wup[:, 128 * (1 + m):128 * (2 + m)] = w_up1
    w1 = (np.asarray(inputs["mlp_w1"]) / math.sqrt(N_RADIAL)).astype(np.float16)
    w2 = (np.asarray(inputs["mlp_w2"]) / math.sqrt(HIDDEN)).astype(np.float16)
    w3 = (np.asarray(inputs["mlp_w3"]) / math.sqrt(HIDDEN)).astype(np.float16)
    w4 = np.asarray(inputs["mlp_w4"]) / math.sqrt(HIDDEN)
    w4 = w4 * np.concatenate([np.full(128, c), np.full(128, c3),
                              np.full(128, c), np.full(128, c)])
    w4 = w4.astype(np.float16)
    wlin = np.zeros((P, 512), np.float16)
    lin0 = (np.asarray(inputs["W_lin0"]) * inv2 / 10.0).astype(np.float16)
    lin1 = (np.asarray(inputs["W_lin1"]) * inv2 / 10.0).astype(np.float16)
    wlin[:, 0:128] = lin0[:128]
    wlin[:, 128:256] = lin0[128:]
    wlin[:, 256:384] = lin1[:128]
    wlin[:, 384:512] = lin1[128:]

    iota = np.tile(np.arange(P, dtype=np.float32), (P, 1))
    ident = np.eye(P, dtype=np.float16)

    core_of = rcv // NODES_PER_CORE
    tile_of = (rcv % NODES_PER_CORE) // P
    sizes = np.zeros((NCORES, TILES_PER_CORE), np.int64)
    np.add.at(sizes, (core_of, tile_of), 1)
    c_prof = tuple(max(1, int(math.ceil(sizes[:, t].max() / P)))
                   for t in range(TILES_PER_CORE))
    nch = sum(c_prof)
    ne_pad = nch * P

    order = np.lexsort((rcv, tile_of, core_of))
    er_all = np.zeros((NCORES, ne_pad, 6), np.float32)
    eft_all = np.zeros((NCORES, N_RADIAL, ne_pad), np.float16)

    starts = np.concatenate([[0], np.cumsum(np.asarray(c_prof)) * P])[:-1]
    flat_sizes = sizes.reshape(-1)
    run_start = np.concatenate([[0], np.cumsum(flat_sizes)])[:-1].reshape(
        NCORES, TILES_PER_CORE)

    for cidx in range(NCORES):
        for t in range(TILES_PER_CORE):
            n = int(sizes[cidx, t])
            if n == 0:
                continue
            e = order[run_start[cidx, t]:run_start[cidx, t] + n]
            s0 = int(starts[t])
            er_all[cidx, s0:s0 + n, 0] = snd[e].astype(np.int32).view(np.float32)
            er_all[cidx, s0:s0 + n, 1] = (rcv[e] % NODES_PER_CORE) - t * P
            er_all[cidx, s0:s0 + n, 2:6] = ea[e]
            eft_all[cidx, :, s0:s0 + n] = ef[e].astype(np.float16).T

    common = dict(nfT=nfT, wup=wup, w1d=w1, w2d=w2, w3d=w3, w4d=w4,
                  wlind=wlin, iotad=iota, identd=ident)
    in_maps = []
    for cidx in range(NCORES):
        m = dict(common)
        m.update(erd=er_all[cidx], eftd=eft_all[cidx])
        in_maps.append(m)
    return c_prof, in_maps


def kernel(**inputs):
    from concourse.bass_utils import run_bass_kernel_spmd

    c_prof, in_maps = _host_prep(inputs)
    if c_prof not in _CACHE:
        _CACHE[c_prof] = _build(c_prof)
    nc = _CACHE[c_prof]

    trace = bool(os.environ.get("KERNEL_TRACE"))
    if trace:
        import sys, types
        import concourse.bass_utils as bu
        try:
            import antenv.axon_hooks  # noqa
        except ImportError:
            import trn_agent_boot.trn_boot as tb
            hooks = types.ModuleType("antenv.axon_hooks")
            hk = tb._ntff_profile_via_ctypes("/opt/axon/libaxon_pjrt.so")
            hooks.get_axon_ntff_profile_hook = lambda: hk
            hooks.set_axon_ntff_profile_hook = lambda h: None
            sys.modules["antenv.axon_hooks"] = hooks
        bu.upload_artifacts = lambda d: d

    res = run_bass_kernel_spmd(nc, in_maps, list(range(NCORES)), trace=trace)
    if trace and res.exec_time_ns is not None:
        print(f"HW exec time: {res.exec_time_ns} ns")
        if res.instructions_and_trace:
            print(f"trace: {res.instructions_and_trace[1]}")

    out = np.empty((N_NODES, 512), np.float32)
    for cidx in range(NCORES):
        lo = cidx * NODES_PER_CORE
        hi = min((cidx + 1) * NODES_PER_CORE, N_NODES)
        if lo >= N_NODES:
            break
        out[lo:hi] = res.results[cidx]["outd"][:hi - lo]
    return out


# revision 13
# speedup vs baseline: 1.4359x; 1.1303x over previous
"""E3nn interaction (gnn message passing) Bass kernel for 8 Trainium2 cores.

Strategy: edges are sorted by receiver and partitioned so core i owns the
segment-sum for nodes [2560*i, 2560*(i+1)).  Each core redundantly computes
the up-projected node table (fp16) into its own DRAM from host-pretransposed
features, then streams its edge chunks: indirect-gather of sender rows,
radial MLP on the tensor engine, fused per-edge tensor-product ops on
DVE/ACT/POOL, and a one-hot matmul scatter accumulating messages in PSUM.
Per 128-node tile the accumulator is transposed on PE and the final linear
is applied, writing the core's output rows directly.
"""
import math
import os
import numpy as np

N_NODES = 20000
N_EDGES = 200000
MUL = 128
P = 128
NCORES = 8
TILES_PER_CORE = 20
NODES_PER_CORE = TILES_PER_CORE * P          # 2560
NODE_PAD = NCORES * NODES_PER_CORE           # 20480
N_NODE_TILES = NODE_PAD // P                 # 160
N_RADIAL = 8
HIDDEN = 64

_CACHE = {}


def _build(c_prof):
    import concourse.bacc as bacc
    import concourse.bass as bass
    import concourse.tile as tile
    from concourse import mybir

    f16, f32, i32 = mybir.dt.float16, mybir.dt.float32, mybir.dt.int32
    MUL_ = mybir.AluOpType.mult
    ADD_ = mybir.AluOpType.add
    EQ_ = mybir.AluOpType.is_equal
    SILU = mybir.ActivationFunctionType.Silu
    COPY = mybir.ActivationFunctionType.Copy

    nch = sum(c_prof)
    ne_pad = nch * P

    nc = bacc.Bacc()
    nfT = nc.declare_dram_parameter("nfT", [512, NODE_PAD], f16, isOutput=False)
    wup = nc.declare_dram_parameter("wup", [P, 512], f16, isOutput=False)
    w1d = nc.declare_dram_parameter("w1d", [N_RADIAL, HIDDEN], f16, isOutput=False)
    w2d = nc.declare_dram_parameter("w2d", [HIDDEN, HIDDEN], f16, isOutput=False)
    w3d = nc.declare_dram_parameter("w3d", [HIDDEN, HIDDEN], f16, isOutput=False)
    w4d = nc.declare_dram_parameter("w4d", [HIDDEN, 512], f16, isOutput=False)
    wlind = nc.declare_dram_parameter("wlind", [P, 512], f16, isOutput=False)
    iotad = nc.declare_dram_parameter("iotad", [P, P], f16, isOutput=False)
    identd = nc.declare_dram_parameter("identd", [P, P], f16, isOutput=False)
    erd = nc.declare_dram_parameter("erd", [ne_pad, 6], f32, isOutput=False)
    eftd = nc.declare_dram_parameter("eftd", [N_RADIAL, ne_pad], f16, isOutput=False)
    outd = nc.declare_dram_parameter("outd", [NODES_PER_CORE, 512], f32, isOutput=True)

    with tile.TileContext(nc) as tc:
        with tc.tile_pool(name="const", bufs=1) as cp, \
             tc.tile_pool(name="dram", bufs=1, space="DRAM") as dp, \
             tc.tile_pool(name="upsb", bufs=10) as up_sb, \
             tc.tile_pool(name="edge", bufs=8) as ep, \
             tc.tile_pool(name="prod", bufs=6) as pp, \
             tc.tile_pool(name="flush", bufs=3) as fp, \
             tc.tile_pool(name="psA", bufs=1, space="PSUM") as psA, \
             tc.tile_pool(name="psW", bufs=3, space="PSUM") as psW, \
             tc.tile_pool(name="psH", bufs=2, space="PSUM") as psH, \
             tc.tile_pool(name="psF", bufs=1, space="PSUM") as psF:

            table = dp.tile([NODE_PAD, 512], f16)

            wup_t = cp.tile([P, 512], f16)
            nc.sync.dma_start(out=wup_t[:], in_=wup[:])
            w1_t = cp.tile([N_RADIAL, HIDDEN], f16)
            nc.sync.dma_start(out=w1_t[:], in_=w1d[:])
            w2_t = cp.tile([HIDDEN, HIDDEN], f16)
            nc.sync.dma_start(out=w2_t[:], in_=w2d[:])
            w3_t = cp.tile([HIDDEN, HIDDEN], f16)
            nc.sync.dma_start(out=w3_t[:], in_=w3d[:])
            w4_t = cp.tile([HIDDEN, 512], f16)
            nc.sync.dma_start(out=w4_t[:], in_=w4d[:])
            wlin_t = cp.tile([P, 512], f16)
            nc.sync.dma_start(out=wlin_t[:], in_=wlind[:])
            iota_t = cp.tile([P, P], f16)
            nc.sync.dma_start(out=iota_t[:], in_=iotad[:])
            ident_t = cp.tile([P, P], f16)
            nc.sync.dma_start(out=ident_t[:], in_=identd[:])
            zt = cp.tile([P, P], f16)
            nc.vector.memset(zt[:], 0.0)

            # ---- Phase A: up-projection table (all nodes, replicated) ----
            for nt in range(N_NODE_TILES):
                r0 = nt * P
                if nt % 2 == 0:
                    xT8 = up_sb.tile([P, 8 * P], f16, tag="xT8")
                    nc.sync.dma_start(
                        out=xT8[:].rearrange("p (b n) -> p b n", n=2 * P),
                        in_=nfT[:, r0:r0 + 2 * P].rearrange(
                            "(b p) n -> p b n", p=P))
                off = (nt % 2) * P
                ups = psW.tile([P, 512], f32, tag="w512", name="ups")
                for b in range(4):
                    nc.tensor.matmul(
                        out=ups[:, b * P:(b + 1) * P],
                        lhsT=xT8[:, b * 2 * P + off:b * 2 * P + off + P],
                        rhs=wup_t[:, b * P:(b + 1) * P], start=True, stop=True)
                urow = up_sb.tile([P, 512], f16, tag="urow")
                if nt % 2 == 0:
                    nc.vector.tensor_copy(out=urow[:], in_=ups[:])
                else:
                    nc.scalar.copy(out=urow[:], in_=ups[:])
                nc.sync.dma_start(out=table[r0:r0 + P, :], in_=urow[:])

            # ---- Phase B: edge chunks ----
            ci_global = 0
            for t in range(TILES_PER_CORE):
                n_chunks = c_prof[t]
                acc = psA.tile([P, 1024], f32, tag="acc")
                nc.tensor.matmul(out=acc[:, 0:512], lhsT=zt[:], rhs=wup_t[:],
                                 start=True, stop=True, skip_group_check=True)
                nc.tensor.matmul(out=acc[:, 512:1024], lhsT=zt[:], rhs=wup_t[:],
                                 start=True, stop=True, skip_group_check=True)
                for ci in range(n_chunks):
                    e0 = ci_global * P
                    ci_global += 1
                    k = ci % 2
                    if k == 0:
                        npair = min(2, n_chunks - ci)
                        er2 = ep.tile([P, 12], f32, tag="er2")
                        nc.sync.dma_start(
                            out=er2[:, 0:6 * npair].rearrange(
                                "p (k c) -> p k c", c=6),
                            in_=erd[e0:e0 + npair * P, :].rearrange(
                                "(k p) c -> p k c", p=P))
                        eft2 = ep.tile([N_RADIAL, 2 * P], f16, tag="eft2")
                        nc.sync.dma_start(out=eft2[:, 0:npair * P],
                                          in_=eftd[:, e0:e0 + npair * P])
                    gidx = er2[:, 6 * k:6 * k + 1].bitcast(i32)
                    rloc = er2[:, 6 * k + 1:6 * k + 2]
                    at = er2[:, 6 * k + 2:6 * k + 6]
                    eft = eft2[:, k * P:(k + 1) * P]
                    g = ep.tile([P, 512], f16, tag="g")
                    nc.gpsimd.indirect_dma_start(
                        out=g[:], out_offset=None, in_=table[:],
                        in_offset=bass.IndirectOffsetOnAxis(ap=gidx, axis=0))

                    # radial MLP (PE + ACT silu)
                    hps = psH.tile([HIDDEN, 3 * P], f32, tag="hps")
                    nc.tensor.matmul(out=hps[:, 0:P], lhsT=w1_t[:], rhs=eft,
                                     start=True, stop=True)
                    h1 = pp.tile([HIDDEN, P], f16, tag="h1")
                    nc.scalar.activation(out=h1[:], in_=hps[:, 0:P], func=SILU)
                    nc.tensor.matmul(out=hps[:, P:2 * P], lhsT=w2_t[:], rhs=h1[:],
                                     start=True, stop=True)
                    h2 = pp.tile([HIDDEN, P], f16, tag="h2")
                    nc.scalar.activation(out=h2[:], in_=hps[:, P:2 * P], func=SILU)
                    nc.tensor.matmul(out=hps[:, 2 * P:3 * P], lhsT=w3_t[:], rhs=h2[:],
                                     start=True, stop=True)
                    h3 = pp.tile([HIDDEN, P], f16, tag="h3")
                    nc.scalar.activation(out=h3[:], in_=hps[:, 2 * P:3 * P], func=SILU)
                    tpw = psW.tile([P, 512], f32, tag="w512", name="tpw")
                    nc.tensor.matmul(out=tpw[:], lhsT=h3[:], rhs=w4_t[:],
                                     start=True, stop=True)
                    wt = pp.tile([P, 512], f16, tag="wt")   # w0|w1|w2|w3
                    if ci % 2 == 0:
                        nc.vector.tensor_copy(out=wt[:], in_=tpw[:])
                    else:
                        nc.scalar.copy(out=wt[:], in_=tpw[:])

                    # per-edge tensor product -> mji [128e, 1024]
                    # layout: [m0a | m0b | m1a(3) | m1b(3)]
                    # gather-only-dependent ops first (keep DVE fed while
                    # the MLP chain produces wt)
                    y0 = at[:, 0:1]
                    oh = pp.tile([P, P], f16, tag="oh")
                    nc.vector.tensor_scalar(
                        out=oh[:], in0=iota_t[:], scalar1=rloc, scalar2=None,
                        op0=EQ_)
                    vy = pp.tile([P, 3 * P], f16, tag="vy")
                    nc.vector.tensor_tensor(
                        out=vy[:].rearrange("p (m u) -> p m u", u=P),
                        in0=g[:, P:4 * P].rearrange("p (m u) -> p m u", u=P),
                        in1=at[:, 1:4].to_broadcast([P, 3, P]),
                        op=MUL_)
                    d01 = pp.tile([P, P], f16, tag="d01")
                    nc.vector.tensor_add(out=d01[:], in0=vy[:, 0:P],
                                         in1=vy[:, P:2 * P])
                    d2 = pp.tile([P, P], f16, tag="d2")
                    nc.gpsimd.tensor_add(out=d2[:], in0=d01[:], in1=vy[:, 2 * P:3 * P])
                    mji = pp.tile([P, 1024], f16, tag="mji")
                    nc.vector.scalar_tensor_tensor(
                        out=mji[:, 0:P], in0=g[:, 0:P], scalar=y0, in1=wt[:, 0:P],
                        op0=MUL_, op1=MUL_)
                    sw2 = pp.tile([P, P], f16, tag="sw2")
                    nc.vector.tensor_mul(out=sw2[:], in0=g[:, 0:P],
                                         in1=wt[:, 2 * P:3 * P])
                    for m in range(3):
                        nc.scalar.activation(
                            out=mji[:, (2 + m) * P:(3 + m) * P], in_=sw2[:],
                            func=COPY, scale=at[:, 1 + m:2 + m])
                    w3y0 = pp.tile([P, P], f16, tag="w3y0")
                    nc.vector.tensor_scalar(
                        out=w3y0[:], in0=wt[:, 3 * P:4 * P], scalar1=y0,
                        scalar2=None, op0=MUL_)
                    for m in range(3):
                        nc.vector.tensor_mul(
                            out=mji[:, (5 + m) * P:(6 + m) * P],
                            in0=g[:, (1 + m) * P:(2 + m) * P], in1=w3y0[:])
                    nc.gpsimd.tensor_mul(out=mji[:, P:2 * P], in0=d2[:],
                                         in1=wt[:, P:2 * P])

                    nc.tensor.matmul(out=acc[:, 0:512], lhsT=oh[:],
                                     rhs=mji[:, 0:512], start=False,
                                     stop=(ci == n_chunks - 1),
                                     skip_group_check=True)
                    nc.tensor.matmul(out=acc[:, 512:1024], lhsT=oh[:],
                                     rhs=mji[:, 512:1024], start=False,
                                     stop=(ci == n_chunks - 1),
                                     skip_group_check=True)

                # ---- flush node tile t ----
                msg = fp.tile([P, 1024], f16, tag="msg")
                nc.vector.tensor_copy(out=msg[:, 0:512], in_=acc[:, 0:512])
                nc.scalar.copy(out=msg[:, 512:1024], in_=acc[:, 512:1024])
                psT = psF.tile([P, 1024], f16, tag="psTfin", name="psT")
                for b in range(8):
                    nc.tensor.transpose(
                        out=psT[:, b * P:(b + 1) * P],
                        in_=msg[:, b * P:(b + 1) * P], identity=ident_t[:])
                msgT = fp.tile([P, 1024], f16, tag="msgT")
                nc.vector.tensor_copy(out=msgT[:, 0:512], in_=psT[:, 0:512])
                nc.scalar.copy(out=msgT[:, 512:1024], in_=psT[:, 512:1024])
                fin = psF.tile([P, 512], f32, tag="psTfin", name="fin")
                nc.tensor.matmul(out=fin[:], lhsT=zt[:], rhs=wup_t[:],
                                 start=True, stop=True, skip_group_check=True)
                nc.tensor.matmul(out=fin[:, 0:P], lhsT=msgT[:, 0:P],
                                 rhs=wlin_t[:, 0:P], start=False, stop=False,
                                 skip_group_check=True)
                nc.tensor.matmul(out=fin[:, 0:P], lhsT=msgT[:, P:2 * P],
                                 rhs=wlin_t[:, P:2 * P], start=False, stop=True,
                                 skip_group_check=True)
                for m in range(3):
                    nc.tensor.matmul(
                        out=fin[:, (1 + m) * P:(2 + m) * P],
                        lhsT=msgT[:, (2 + m) * P:(3 + m) * P],
                        rhs=wlin_t[:, 2 * P:3 * P], start=False, stop=False,
                        skip_group_check=True)
                    nc.tensor.matmul(
                        out=fin[:, (1 + m) * P:(2 + m) * P],
                        lhsT=msgT[:, (5 + m) * P:(6 + m) * P],
                        rhs=wlin_t[:, 3 * P:4 * P], start=False, stop=True,
                        skip_group_check=True)
                ot = fp.tile([P, 512], f32, tag="ot")
                nc.vector.tensor_copy(out=ot[:, 0:P], in_=fin[:, 0:P])
                for m in range(3):
                    dst = ot[:, P + m:512:3]
                    if m == 0:
                        nc.vector.tensor_copy(out=dst, in_=fin[:, P:2 * P])
                    elif m == 1:
                        nc.scalar.copy(out=dst, in_=fin[:, 2 * P:3 * P])
                    else:
                        nc.vector.tensor_copy(out=dst, in_=fin[:, 3 * P:4 * P])
                nc.sync.dma_start(out=outd[t * P:(t + 1) * P, :], in_=ot[:])

    nc.compile()
    return nc


def _host_prep(inputs):
    nf = np.asarray(inputs["node_feats"], dtype=np.float32)
    ea = np.asarray(inputs["edge_attrs"], dtype=np.float32)
    ef = np.asarray(inputs["edge_feats"], dtype=np.float32)
    snd = np.asarray(inputs["sender"]).astype(np.int64)
    rcv = np.asarray(inputs["receiver"]).astype(np.int64)

    inv = 1.0 / math.sqrt(MUL)
    inv2 = 1.0 / math.sqrt(2 * MUL)
    c = 1.0 / math.sqrt(MUL)
    c3 = 1.0 / math.sqrt(3.0 * MUL)

    # node feats fp16, transposed block-major: row b*128+ch, col n
    s = nf[:, :MUL]
    v = nf[:, MUL:].reshape(-1, MUL, 3)
    nfT = np.zeros((512, NODE_PAD), np.float16)
    nfT[0:128, :N_NODES] = s.T
    for m in range(3):
        nfT[128 * (1 + m):128 * (2 + m), :N_NODES] = v[:, :, m].T

    wup = np.zeros((P, 512), np.float16)
    wup[:, 0:128] = (np.asarray(inputs["W_up0"]) * inv).astype(np.float16)
    w_up1 = (np.asarray(inputs["W_up1"]) * inv).astype(np.float16)
    for m in range(3):
        wup[:, 128 * (1 + m):128 * (2 + m)] = w_up1
    w1 = (np.asarray(inputs["mlp_w1"]) / math.sqrt(N_RADIAL)).astype(np.float16)
    w2 = (np.asarray(inputs["mlp_w2"]) / math.sqrt(HIDDEN)).astype(np.float16)
    w3 = (np.asarray(inputs["mlp_w3"]) / math.sqrt(HIDDEN)).astype(np.float16)
    w4 = np.asarray(inputs["mlp_w4"]) / math.sqrt(HIDDEN)
    w4 = w4 * np.concatenate([np.full(128, c), np.full(128, c3),
                              np.full(128, c), np.full(128, c)])
    w4 = w4.astype(np.float16)
    wlin = np.zeros((P, 512), np.float16)
    lin0 = (np.asarray(inputs["W_lin0"]) * inv2 / 10.0).astype(np.float16)
    lin1 = (np.asarray(inputs["W_lin1"]) * inv2 / 10.0).astype(np.float16)
    wlin[:, 0:128] = lin0[:128]
    wlin[:, 128:256] = lin0[128:]
    wlin[:, 256:384] = lin1[:128]
    wlin[:, 384:512] = lin1[128:]

    iota = np.tile(np.arange(P, dtype=np.float16), (P, 1))
    ident = np.eye(P, dtype=np.float16)

    core_of = rcv // NODES_PER_CORE
    tile_of = (rcv % NODES_PER_CORE) // P
    sizes = np.zeros((NCORES, TILES_PER_CORE), np.int64)
    np.add.at(sizes, (core_of, tile_of), 1)
    c_prof = tuple(max(1, int(math.ceil(sizes[:, t].max() / P)))
                   for t in range(TILES_PER_CORE))
    nch = sum(c_prof)
    ne_pad = nch * P

    order = np.lexsort((rcv, tile_of, core_of))
    er_all = np.zeros((NCORES, ne_pad, 6), np.float32)
    eft_all = np.zeros((NCORES, N_RADIAL, ne_pad), np.float16)

    starts = np.concatenate([[0], np.cumsum(np.asarray(c_prof)) * P])[:-1]
    flat_sizes = sizes.reshape(-1)
    run_start = np.concatenate([[0], np.cumsum(flat_sizes)])[:-1].reshape(
        NCORES, TILES_PER_CORE)

    for cidx in range(NCORES):
        for t in range(TILES_PER_CORE):
            n = int(sizes[cidx, t])
            if n == 0:
                continue
            e = order[run_start[cidx, t]:run_start[cidx, t] + n]
            s0 = int(starts[t])
            er_all[cidx, s0:s0 + n, 0] = snd[e].astype(np.int32).view(np.float32)
            er_all[cidx, s0:s0 + n, 1] = (rcv[e] % NODES_PER_CORE) - t * P
            er_all[cidx, s0:s0 + n, 2:6] = ea[e]
            eft_all[cidx, :, s0:s0 + n] = ef[e].astype(np.float16).T

    common = dict(nfT=nfT, wup=wup, w1d=w1, w2d=w2, w3d=w3, w4d=w4,
                  wlind=wlin, iotad=iota, identd=ident)
    in_maps = []
    for cidx in range(NCORES):
        m = dict(common)
        m.update(erd=er_all[cidx], eftd=eft_all[cidx])
        in_maps.append(m)
    return c_prof, in_maps


def kernel(**inputs):
    from concourse.bass_utils import run_bass_kernel_spmd

    c_prof, in_maps = _host_prep(inputs)
    if c_prof not in _CACHE:
        _CACHE[c_prof] = _build(c_prof)
    nc = _CACHE[c_prof]

    trace = bool(os.environ.get("KERNEL_TRACE"))
    if trace:
        import sys, types
        import concourse.bass_utils as bu
        try:
            import antenv.axon_hooks  # noqa
        except ImportError:
            import trn_agent_boot.trn_boot as tb
            hooks = types.ModuleType("antenv.axon_hooks")
            hk = tb._ntff_profile_via_ctypes("/opt/axon/libaxon_pjrt.so")
            hooks.get_axon_ntff_profile_hook = lambda: hk
            hooks.set_axon_ntff_profile_hook = lambda h: None
            sys.modules["antenv.axon_hooks"] = hooks
        bu.upload_artifacts = lambda d: d

    res = run_bass_kernel_spmd(nc, in_maps, list(range(NCORES)), trace=trace)
    if trace and res.exec_time_ns is not None:
        print(f"HW exec time: {res.exec_time_ns} ns")
        if res.instructions_and_trace:
            print(f"trace: {res.instructions_and_trace[1]}")

    out = np.empty((N_NODES, 512), np.float32)
    for cidx in range(NCORES):
        lo = cidx * NODES_PER_CORE
        hi = min((cidx + 1) * NODES_PER_CORE, N_NODES)
        if lo >= N_NODES:
            break
        out[lo:hi] = res.results[cidx]["outd"][:hi - lo]
    return out


# revision 14
# speedup vs baseline: 1.4828x; 1.0326x over previous
"""E3nn interaction (gnn message passing) Bass kernel for 8 Trainium2 cores.

Strategy: edges are sorted by receiver and partitioned so core i owns the
segment-sum for nodes [2560*i, 2560*(i+1)).  Each core redundantly computes
the up-projected node table (fp16) into its own DRAM from host-pretransposed
features, then streams its edge chunks: indirect-gather of sender rows,
radial MLP on the tensor engine, fused per-edge tensor-product ops on
DVE/ACT/POOL, and a one-hot matmul scatter accumulating messages in PSUM.
Per 128-node tile the accumulator is transposed on PE and the final linear
is applied, writing the core's output rows directly.
"""
import math
import os
import numpy as np

N_NODES = 20000
N_EDGES = 200000
MUL = 128
P = 128
NCORES = 8
TILES_PER_CORE = 20
NODES_PER_CORE = TILES_PER_CORE * P          # 2560
NODE_PAD = NCORES * NODES_PER_CORE           # 20480
N_NODE_TILES = NODE_PAD // P                 # 160
N_RADIAL = 8
HIDDEN = 64

_CACHE = {}


def _build(c_prof):
    import concourse.bacc as bacc
    import concourse.bass as bass
    import concourse.tile as tile
    from concourse import mybir

    f16, f32, i32 = mybir.dt.float16, mybir.dt.float32, mybir.dt.int32
    MUL_ = mybir.AluOpType.mult
    ADD_ = mybir.AluOpType.add
    EQ_ = mybir.AluOpType.is_equal
    SILU = mybir.ActivationFunctionType.Silu
    COPY = mybir.ActivationFunctionType.Copy

    nch = sum(c_prof)
    ne_pad = nch * P

    nc = bacc.Bacc()
    nfT = nc.declare_dram_parameter("nfT", [512, NODE_PAD], f16, isOutput=False)
    wup = nc.declare_dram_parameter("wup", [P, 512], f16, isOutput=False)
    w1d = nc.declare_dram_parameter("w1d", [N_RADIAL, HIDDEN], f16, isOutput=False)
    w2d = nc.declare_dram_parameter("w2d", [HIDDEN, HIDDEN], f16, isOutput=False)
    w3d = nc.declare_dram_parameter("w3d", [HIDDEN, HIDDEN], f16, isOutput=False)
    w4d = nc.declare_dram_parameter("w4d", [HIDDEN, 512], f16, isOutput=False)
    wlind = nc.declare_dram_parameter("wlind", [P, 512], f16, isOutput=False)
    iotad = nc.declare_dram_parameter("iotad", [P, P], f16, isOutput=False)
    identd = nc.declare_dram_parameter("identd", [P, P], f16, isOutput=False)
    erd = nc.declare_dram_parameter("erd", [ne_pad, 6], f32, isOutput=False)
    eftd = nc.declare_dram_parameter("eftd", [N_RADIAL, ne_pad], f16, isOutput=False)
    outd = nc.declare_dram_parameter("outd", [NODES_PER_CORE, 512], f32, isOutput=True)

    with tile.TileContext(nc) as tc:
        with tc.tile_pool(name="const", bufs=1) as cp, \
             tc.tile_pool(name="dram", bufs=1, space="DRAM") as dp, \
             tc.tile_pool(name="upsb", bufs=6) as up_sb, \
             tc.tile_pool(name="edge", bufs=8) as ep, \
             tc.tile_pool(name="prod", bufs=6) as pp, \
             tc.tile_pool(name="flush", bufs=3) as fp, \
             tc.tile_pool(name="psA", bufs=1, space="PSUM") as psA, \
             tc.tile_pool(name="psW", bufs=3, space="PSUM") as psW, \
             tc.tile_pool(name="psH", bufs=2, space="PSUM") as psH, \
             tc.tile_pool(name="psF", bufs=1, space="PSUM") as psF:

            table = dp.tile([NODE_PAD, 512], f16)

            wup_t = cp.tile([P, 512], f16)
            nc.sync.dma_start(out=wup_t[:], in_=wup[:])
            w1_t = cp.tile([N_RADIAL, HIDDEN], f16)
            nc.sync.dma_start(out=w1_t[:], in_=w1d[:])
            w2_t = cp.tile([HIDDEN, HIDDEN], f16)
            nc.sync.dma_start(out=w2_t[:], in_=w2d[:])
            w3_t = cp.tile([HIDDEN, HIDDEN], f16)
            nc.sync.dma_start(out=w3_t[:], in_=w3d[:])
            w4_t = cp.tile([HIDDEN, 512], f16)
            nc.sync.dma_start(out=w4_t[:], in_=w4d[:])
            wlin_t = cp.tile([P, 512], f16)
            nc.sync.dma_start(out=wlin_t[:], in_=wlind[:])
            iota_t = cp.tile([P, P], f16)
            nc.sync.dma_start(out=iota_t[:], in_=iotad[:])
            ident_t = cp.tile([P, P], f16)
            nc.sync.dma_start(out=ident_t[:], in_=identd[:])
            zt = cp.tile([P, P], f16)
            nc.vector.memset(zt[:], 0.0)

            # ---- Phase A: up-projection table (all nodes, replicated) ----
            for nt in range(N_NODE_TILES):
                r0 = nt * P
                if nt % 4 == 0:
                    xT16 = up_sb.tile([P, 16 * P], f16, tag="xT16")
                    nc.sync.dma_start(
                        out=xT16[:].rearrange("p (b n) -> p b n", n=4 * P),
                        in_=nfT[:, r0:r0 + 4 * P].rearrange(
                            "(b p) n -> p b n", p=P))
                off = (nt % 4) * P
                ups = psW.tile([P, 512], f32, tag="w512", name="ups")
                for b in range(4):
                    nc.tensor.matmul(
                        out=ups[:, b * P:(b + 1) * P],
                        lhsT=xT16[:, b * 4 * P + off:b * 4 * P + off + P],
                        rhs=wup_t[:, b * P:(b + 1) * P], start=True, stop=True)
                if nt % 2 == 0:
                    urow2 = up_sb.tile([P, 1024], f16, tag="urow2")
                half = (nt % 2) * 512
                if nt % 2 == 0:
                    nc.vector.tensor_copy(out=urow2[:, half:half + 512], in_=ups[:])
                else:
                    nc.scalar.copy(out=urow2[:, half:half + 512], in_=ups[:])
                if nt % 2 == 1:
                    nc.sync.dma_start(
                        out=table[r0 - P:r0 + P, :].rearrange(
                            "(k p) c -> p k c", p=P),
                        in_=urow2[:].rearrange("p (k c) -> p k c", c=512))

            # ---- Phase B: edge chunks ----
            ci_global = 0
            for t in range(TILES_PER_CORE):
                n_chunks = c_prof[t]
                acc = psA.tile([P, 1024], f32, tag="acc")
                nc.tensor.matmul(out=acc[:, 0:512], lhsT=zt[:], rhs=wup_t[:],
                                 start=True, stop=True, skip_group_check=True)
                nc.tensor.matmul(out=acc[:, 512:1024], lhsT=zt[:], rhs=wup_t[:],
                                 start=True, stop=True, skip_group_check=True)
                for ci in range(n_chunks):
                    e0 = ci_global * P
                    ci_global += 1
                    k = ci % 2
                    if k == 0:
                        npair = min(2, n_chunks - ci)
                        er2 = ep.tile([P, 12], f32, tag="er2")
                        nc.sync.dma_start(
                            out=er2[:, 0:6 * npair].rearrange(
                                "p (k c) -> p k c", c=6),
                            in_=erd[e0:e0 + npair * P, :].rearrange(
                                "(k p) c -> p k c", p=P))
                        eft2 = ep.tile([N_RADIAL, 2 * P], f16, tag="eft2")
                        nc.sync.dma_start(out=eft2[:, 0:npair * P],
                                          in_=eftd[:, e0:e0 + npair * P])
                    gidx = er2[:, 6 * k:6 * k + 1].bitcast(i32)
                    rloc = er2[:, 6 * k + 1:6 * k + 2]
                    at = er2[:, 6 * k + 2:6 * k + 6]
                    eft = eft2[:, k * P:(k + 1) * P]
                    g = ep.tile([P, 512], f16, tag="g")
                    nc.gpsimd.indirect_dma_start(
                        out=g[:], out_offset=None, in_=table[:],
                        in_offset=bass.IndirectOffsetOnAxis(ap=gidx, axis=0))

                    # radial MLP (PE + ACT silu)
                    hps = psH.tile([HIDDEN, 3 * P], f32, tag="hps")
                    nc.tensor.matmul(out=hps[:, 0:P], lhsT=w1_t[:], rhs=eft,
                                     start=True, stop=True)
                    h1 = pp.tile([HIDDEN, P], f16, tag="h1")
                    nc.scalar.activation(out=h1[:], in_=hps[:, 0:P], func=SILU)
                    nc.tensor.matmul(out=hps[:, P:2 * P], lhsT=w2_t[:], rhs=h1[:],
                                     start=True, stop=True)
                    h2 = pp.tile([HIDDEN, P], f16, tag="h2")
                    nc.scalar.activation(out=h2[:], in_=hps[:, P:2 * P], func=SILU)
                    nc.tensor.matmul(out=hps[:, 2 * P:3 * P], lhsT=w3_t[:], rhs=h2[:],
                                     start=True, stop=True)
                    h3 = pp.tile([HIDDEN, P], f16, tag="h3")
                    nc.scalar.activation(out=h3[:], in_=hps[:, 2 * P:3 * P], func=SILU)
                    tpw = psW.tile([P, 512], f32, tag="w512", name="tpw")
                    nc.tensor.matmul(out=tpw[:], lhsT=h3[:], rhs=w4_t[:],
                                     start=True, stop=True)
                    wt = pp.tile([P, 512], f16, tag="wt")   # w0|w1|w2|w3
                    if ci % 2 == 0:
                        nc.vector.tensor_copy(out=wt[:], in_=tpw[:])
                    else:
                        nc.scalar.copy(out=wt[:], in_=tpw[:])

                    # per-edge tensor product -> mji [128e, 1024]
                    # layout: [m0a | m0b | m1a(3) | m1b(3)]
                    # gather-only-dependent ops first (keep DVE fed while
                    # the MLP chain produces wt)
                    y0 = at[:, 0:1]
                    oh = pp.tile([P, P], f16, tag="oh")
                    nc.vector.tensor_scalar(
                        out=oh[:], in0=iota_t[:], scalar1=rloc, scalar2=None,
                        op0=EQ_)
                    vy = pp.tile([P, 3 * P], f16, tag="vy")
                    nc.vector.tensor_tensor(
                        out=vy[:].rearrange("p (m u) -> p m u", u=P),
                        in0=g[:, P:4 * P].rearrange("p (m u) -> p m u", u=P),
                        in1=at[:, 1:4].to_broadcast([P, 3, P]),
                        op=MUL_)
                    d01 = pp.tile([P, P], f16, tag="d01")
                    nc.vector.tensor_add(out=d01[:], in0=vy[:, 0:P],
                                         in1=vy[:, P:2 * P])
                    d2 = pp.tile([P, P], f16, tag="d2")
                    nc.gpsimd.tensor_add(out=d2[:], in0=d01[:], in1=vy[:, 2 * P:3 * P])
                    mji = pp.tile([P, 1024], f16, tag="mji")
                    nc.vector.scalar_tensor_tensor(
                        out=mji[:, 0:P], in0=g[:, 0:P], scalar=y0, in1=wt[:, 0:P],
                        op0=MUL_, op1=MUL_)
                    sw2 = pp.tile([P, P], f16, tag="sw2")
                    nc.vector.tensor_mul(out=sw2[:], in0=g[:, 0:P],
                                         in1=wt[:, 2 * P:3 * P])
                    for m in range(3):
                        nc.scalar.activation(
                            out=mji[:, (2 + m) * P:(3 + m) * P], in_=sw2[:],
                            func=COPY, scale=at[:, 1 + m:2 + m])
                    w3y0 = pp.tile([P, P], f16, tag="w3y0")
                    nc.vector.tensor_scalar(
                        out=w3y0[:], in0=wt[:, 3 * P:4 * P], scalar1=y0,
                        scalar2=None, op0=MUL_)
                    nc.vector.tensor_tensor(
                        out=mji[:, 5 * P:8 * P].rearrange("p (m u) -> p m u", u=P),
                        in0=g[:, P:4 * P].rearrange("p (m u) -> p m u", u=P),
                        in1=w3y0[:].rearrange("p (o u) -> p o u", o=1)
                            .to_broadcast([P, 3, P]),
                        op=MUL_)
                    nc.gpsimd.tensor_mul(out=mji[:, P:2 * P], in0=d2[:],
                                         in1=wt[:, P:2 * P])

                    nc.tensor.matmul(out=acc[:, 0:512], lhsT=oh[:],
                                     rhs=mji[:, 0:512], start=False,
                                     stop=(ci == n_chunks - 1),
                                     skip_group_check=True)
                    nc.tensor.matmul(out=acc[:, 512:1024], lhsT=oh[:],
                                     rhs=mji[:, 512:1024], start=False,
                                     stop=(ci == n_chunks - 1),
                                     skip_group_check=True)

                # ---- flush node tile t ----
                msg = fp.tile([P, 1024], f16, tag="msg")
                nc.vector.tensor_copy(out=msg[:, 0:512], in_=acc[:, 0:512])
                nc.scalar.copy(out=msg[:, 512:1024], in_=acc[:, 512:1024])
                psT = psF.tile([P, 1024], f16, tag="psTfin", name="psT")
                for b in range(8):
                    nc.tensor.transpose(
                        out=psT[:, b * P:(b + 1) * P],
                        in_=msg[:, b * P:(b + 1) * P], identity=ident_t[:])
                msgT = fp.tile([P, 1024], f16, tag="msgT")
                nc.vector.tensor_copy(out=msgT[:, 0:512], in_=psT[:, 0:512])
                nc.scalar.copy(out=msgT[:, 512:1024], in_=psT[:, 512:1024])
                fin = psF.tile([P, 512], f32, tag="psTfin", name="fin")
                nc.tensor.matmul(out=fin[:], lhsT=zt[:], rhs=wup_t[:],
                                 start=True, stop=True, skip_group_check=True)
                nc.tensor.matmul(out=fin[:, 0:P], lhsT=msgT[:, 0:P],
                                 rhs=wlin_t[:, 0:P], start=False, stop=False,
                                 skip_group_check=True)
                nc.tensor.matmul(out=fin[:, 0:P], lhsT=msgT[:, P:2 * P],
                                 rhs=wlin_t[:, P:2 * P], start=False, stop=True,
                                 skip_group_check=True)
                for m in range(3):
                    nc.tensor.matmul(
                        out=fin[:, (1 + m) * P:(2 + m) * P],
                        lhsT=msgT[:, (2 + m) * P:(3 + m) * P],
                        rhs=wlin_t[:, 2 * P:3 * P], start=False, stop=False,
                        skip_group_check=True)
                    nc.tensor.matmul(
                        out=fin[:, (1 + m) * P:(2 + m) * P],
                        lhsT=msgT[:, (5 + m) * P:(6 + m) * P],
                        rhs=wlin_t[:, 3 * P:4 * P], start=False, stop=True,
                        skip_group_check=True)
                ot = fp.tile([P, 512], f32, tag="ot")
                nc.vector.tensor_copy(out=ot[:, 0:P], in_=fin[:, 0:P])
                for m in range(3):
                    dst = ot[:, P + m:512:3]
                    if m == 0:
                        nc.vector.tensor_copy(out=dst, in_=fin[:, P:2 * P])
                    elif m == 1:
                        nc.scalar.copy(out=dst, in_=fin[:, 2 * P:3 * P])
                    else:
                        nc.vector.tensor_copy(out=dst, in_=fin[:, 3 * P:4 * P])
                nc.sync.dma_start(out=outd[t * P:(t + 1) * P, :], in_=ot[:])

    nc.compile()
    return nc


def _host_prep(inputs):
    nf = np.asarray(inputs["node_feats"], dtype=np.float32)
    ea = np.asarray(inputs["edge_attrs"], dtype=np.float32)
    ef = np.asarray(inputs["edge_feats"], dtype=np.float32)
    snd = np.asarray(inputs["sender"]).astype(np.int64)
    rcv = np.asarray(inputs["receiver"]).astype(np.int64)

    inv = 1.0 / math.sqrt(MUL)
    inv2 = 1.0 / math.sqrt(2 * MUL)
    c = 1.0 / math.sqrt(MUL)
    c3 = 1.0 / math.sqrt(3.0 * MUL)

    # node feats fp16, transposed block-major: row b*128+ch, col n
    s = nf[:, :MUL]
    v = nf[:, MUL:].reshape(-1, MUL, 3)
    nfT = np.zeros((512, NODE_PAD), np.float16)
    nfT[0:128, :N_NODES] = s.T
    for m in range(3):
        nfT[128 * (1 + m):128 * (2 + m), :N_NODES] = v[:, :, m].T

    wup = np.zeros((P, 512), np.float16)
    wup[:, 0:128] = (np.asarray(inputs["W_up0"]) * inv).astype(np.float16)
    w_up1 = (np.asarray(inputs["W_up1"]) * inv).astype(np.float16)
    for m in range(3):
        wup[:, 128 * (1 + m):128 * (2 + m)] = w_up1
    w1 = (np.asarray(inputs["mlp_w1"]) / math.sqrt(N_RADIAL)).astype(np.float16)
    w2 = (np.asarray(inputs["mlp_w2"]) / math.sqrt(HIDDEN)).astype(np.float16)
    w3 = (np.asarray(inputs["mlp_w3"]) / math.sqrt(HIDDEN)).astype(np.float16)
    w4 = np.asarray(inputs["mlp_w4"]) / math.sqrt(HIDDEN)
    w4 = w4 * np.concatenate([np.full(128, c), np.full(128, c3),
                              np.full(128, c), np.full(128, c)])
    w4 = w4.astype(np.float16)
    wlin = np.zeros((P, 512), np.float16)
    lin0 = (np.asarray(inputs["W_lin0"]) * inv2 / 10.0).astype(np.float16)
    lin1 = (np.asarray(inputs["W_lin1"]) * inv2 / 10.0).astype(np.float16)
    wlin[:, 0:128] = lin0[:128]
    wlin[:, 128:256] = lin0[128:]
    wlin[:, 256:384] = lin1[:128]
    wlin[:, 384:512] = lin1[128:]

    iota = np.tile(np.arange(P, dtype=np.float16), (P, 1))
    ident = np.eye(P, dtype=np.float16)

    core_of = rcv // NODES_PER_CORE
    tile_of = (rcv % NODES_PER_CORE) // P
    sizes = np.zeros((NCORES, TILES_PER_CORE), np.int64)
    np.add.at(sizes, (core_of, tile_of), 1)
    c_prof = tuple(max(1, int(math.ceil(sizes[:, t].max() / P)))
                   for t in range(TILES_PER_CORE))
    nch = sum(c_prof)
    ne_pad = nch * P

    order = np.lexsort((rcv, tile_of, core_of))
    er_all = np.zeros((NCORES, ne_pad, 6), np.float32)
    eft_all = np.zeros((NCORES, N_RADIAL, ne_pad), np.float16)

    starts = np.concatenate([[0], np.cumsum(np.asarray(c_prof)) * P])[:-1]
    flat_sizes = sizes.reshape(-1)
    run_start = np.concatenate([[0], np.cumsum(flat_sizes)])[:-1].reshape(
        NCORES, TILES_PER_CORE)

    for cidx in range(NCORES):
        for t in range(TILES_PER_CORE):
            n = int(sizes[cidx, t])
            if n == 0:
                continue
            e = order[run_start[cidx, t]:run_start[cidx, t] + n]
            s0 = int(starts[t])
            er_all[cidx, s0:s0 + n, 0] = snd[e].astype(np.int32).view(np.float32)
            er_all[cidx, s0:s0 + n, 1] = (rcv[e] % NODES_PER_CORE) - t * P
            er_all[cidx, s0:s0 + n, 2:6] = ea[e]
            eft_all[cidx, :, s0:s0 + n] = ef[e].astype(np.float16).T

    common = dict(nfT=nfT, wup=wup, w1d=w1, w2d=w2, w3d=w3, w4d=w4,
                  wlind=wlin, iotad=iota, identd=ident)
    in_maps = []
    for cidx in range(NCORES):
        m = dict(common)
        m.update(erd=er_all[cidx], eftd=eft_all[cidx])
        in_maps.append(m)
    return c_prof, in_maps


def kernel(**inputs):
    from concourse.bass_utils import run_bass_kernel_spmd

    c_prof, in_maps = _host_prep(inputs)
    if c_prof not in _CACHE:
        _CACHE[c_prof] = _build(c_prof)
    nc = _CACHE[c_prof]

    trace = bool(os.environ.get("KERNEL_TRACE"))
    if trace:
        import sys, types
        import concourse.bass_utils as bu
        try:
            import antenv.axon_hooks  # noqa
        except ImportError:
            import trn_agent_boot.trn_boot as tb
            hooks = types.ModuleType("antenv.axon_hooks")
            hk = tb._ntff_profile_via_ctypes("/opt/axon/libaxon_pjrt.so")
            hooks.get_axon_ntff_profile_hook = lambda: hk
            hooks.set_axon_ntff_profile_hook = lambda h: None
            sys.modules["antenv.axon_hooks"] = hooks
        bu.upload_artifacts = lambda d: d

    res = run_bass_kernel_spmd(nc, in_maps, list(range(NCORES)), trace=trace)
    if trace and res.exec_time_ns is not None:
        print(f"HW exec time: {res.exec_time_ns} ns")
        if res.instructions_and_trace:
            print(f"trace: {res.instructions_and_trace[1]}")

    out = np.empty((N_NODES, 512), np.float32)
    for cidx in range(NCORES):
        lo = cidx * NODES_PER_CORE
        hi = min((cidx + 1) * NODES_PER_CORE, N_NODES)
        if lo >= N_NODES:
            break
        out[lo:hi] = res.results[cidx]["outd"][:hi - lo]
    return out
